# revision 1
# baseline (speedup 1.0000x reference)
"""Trainium2 Bass kernel for nn_Attention (B=4, S=2048, D=2048, H=16, KV=4, HD=128).

Sharding (8 cores): data-parallel over batch (4) x tensor-parallel over
KV-head-group halves (2). Core c handles batch b=c//2 and q-heads
[8*(c%2), 8*(c%2)+8) == kv groups {2*(c%2), 2*(c%2)+1}. Each core produces a
partial output (its heads' contribution through wo); the host sums the two
partials per batch.

All big matmuls run in float32r (full PE speed, ~1.6e-4 rel err). Attention is
computed transposed (scoresT[k,q]: kT-block stationary, qT moving) so the ACT
exp pass doubles as the PSUM->SBUF move and no probs transposes are needed (no
max subtraction; scores are O(6) here). Softmax denominators come from a
ones-row matmul accumulated in PSUM; normalization multiplies the AV output by
a broadcast reciprocal tile (ones-column x recip-row matmul). AV accumulates in
PSUM (V stationary, probsT moving); the output projection (woT stationary,
attT moving) emits a transposed partial output; host transposes back and sums
core pairs. Copy engines (ACT vs DVE) and PSUM/SBUF pool depths are tuned via
TimelineSim A/B sweeps: ~589us/core, ~1.21x the fp32r PE-work floor.
"""
import numpy as np

B, S, D = 4, 2048, 2048
H, KV, HD = 16, 4, 128
NREP = H // KV
SCALE = float(HD) ** -0.5

SB = S // 128          # 16 s-blocks
KT = D // 128          # 16 contraction tiles for projections
QSB = S // 512         # 4 q-superblocks
HPC = 8                # q heads per core
GPC = 2                # kv groups per core

_compiled = {}


def _build(causal: bool):
    import concourse.bass as bass  # noqa: F401
    import concourse.tile as tile
    from concourse import bacc, mybir
    from concourse.masks import make_identity

    f32 = mybir.dt.float32
    f32r = mybir.dt.float32r
    AF = mybir.ActivationFunctionType
    ALU = mybir.AluOpType

    nc = bacc.Bacc("TRN2")

    xT = nc.dram_tensor("xT", [D, S], f32r, kind="ExternalInput")
    wqT = nc.dram_tensor("wqT", [D, HPC * HD], f32r, kind="ExternalInput")
    wkvT = nc.dram_tensor("wkvT", [D, 2 * GPC * HD], f32r, kind="ExternalInput")
    woT = nc.dram_tensor("woT", [HPC * HD, D], f32r, kind="ExternalInput")
    cosS = nc.dram_tensor("cosS", [128, SB, 64], f32, kind="ExternalInput")
    sinS = nc.dram_tensor("sinS", [128, SB, 64], f32, kind="ExternalInput")
    mtile = nc.dram_tensor("mtile", [128, 128], f32, kind="ExternalInput")
    onest = nc.dram_tensor("onest", [128, 128], f32r, kind="ExternalInput")
    outT = nc.dram_tensor("outT", [D, S], f32, kind="ExternalOutput")

    xT3 = xT.rearrange("(kt p) s -> p kt s", p=128)
    woT3 = woT.rearrange("(h p) d -> p h d", p=128)

    with tile.TileContext(nc) as tc:
        with tc.tile_pool(name="persist", bufs=1) as persist:
            qT = [persist.tile([128, S], f32r, tag=f"qT{h}", name=f"qT{h}") for h in range(HPC)]
            kT = [persist.tile([128, S], f32r, tag=f"kTg{g}", name=f"kTg{g}") for g in range(GPC)]
            vsb = [persist.tile([128, SB, 128], f32r, tag=f"v{g}", name=f"v{g}") for g in range(GPC)]
            msk = persist.tile([128, 128], f32, tag="msk")
            nc.sync.dma_start(out=msk, in_=mtile[:, :])
            ones = persist.tile([128, 128], f32r, tag="ones")
            nc.sync.dma_start(out=ones, in_=onest[:, :])

            # ------------ Stage 1: projections + RoPE + transposes ----------
            s1ctx = tc.tile_pool(name="s1const", bufs=1)
            s1const = s1ctx.__enter__()
            ident_f = s1const.tile([128, 128], f32, tag="identf")
            make_identity(nc, ident_f)
            ident = s1const.tile([128, 128], f32r, tag="ident")
            nc.vector.tensor_copy(out=ident, in_=ident_f)
            cos_t = s1const.tile([128, SB, 64], f32, tag="cos")
            sin_t = s1const.tile([128, SB, 64], f32, tag="sin")
            nc.sync.dma_start(out=cos_t, in_=cosS[:, :, :])
            nc.sync.dma_start(out=sin_t, in_=sinS[:, :, :])

            def proj_pass(wT_ap, e_width, kind, head_base=0):
                nh = e_width // 128
                with tc.tile_pool(name="w1", bufs=1) as wpool, \
                     tc.tile_pool(name="xs1", bufs=2) as xpool, \
                     tc.tile_pool(name="rs1", bufs=2) as rpool, \
                     tc.tile_pool(name="pq1", bufs=3, space="PSUM") as pqp, \
                     tc.tile_pool(name="pt1", bufs=2, space="PSUM") as ptp:
                    wt = wpool.tile([128, KT, e_width], f32r, tag="wt")
                    wT3 = wT_ap.rearrange("(kt p) e -> p kt e", p=128)
                    for kt4 in range(0, KT, 2):
                        nc.sync.dma_start(
                            out=wt[:, kt4:kt4 + 2, :], in_=wT3[:, kt4:kt4 + 2, :])
                    for sb in range(SB):
                        xs = xpool.tile([128, KT, 128], f32r, tag="xs")
                        nc.sync.dma_start(
                            out=xs[:, 0:8, :],
                            in_=xT3[:, 0:8, sb * 128:(sb + 1) * 128])
                        nc.sync.dma_start(
                            out=xs[:, 8:16, :],
                            in_=xT3[:, 8:16, sb * 128:(sb + 1) * 128])
                        ps = pqp.tile([128, e_width], f32, tag="ps")
                        for kt in range(KT):
                            for n0 in range(0, e_width, 512):
                                nw = min(512, e_width - n0)
                                nc.tensor.matmul(
                                    ps[:, n0:n0 + nw], xs[:, kt, :],
                                    wt[:, kt, n0:n0 + nw],
                                    start=(kt == 0), stop=(kt == KT - 1))
                        ps3 = ps.rearrange("p (h d) -> p h d", d=128)
                        nr = GPC if kind == "kv" else nh  # heads that get RoPE
                        if kind == "kv":
                            for g in range(GPC):
                                nc.scalar.copy(
                                    out=vsb[g][:, sb, :], in_=ps3[:, GPC + g, :])
                        rp = rpool.tile([128, HPC, 128], f32r, tag="rope")
                        ev = ps3[:, 0:nr, 0:128:2]
                        od = ps3[:, 0:nr, 1:128:2]
                        cb = cos_t[:, None, sb, :].broadcast_to([128, nr, 64])
                        sn = sin_t[:, None, sb, :].broadcast_to([128, nr, 64])
                        t1 = rpool.tile([128, HPC, 64], f32, tag="t1")
                        t2 = rpool.tile([128, HPC, 64], f32, tag="t2")
                        nc.vector.tensor_tensor(
                            out=t1[:, 0:nr, :], in0=ev, in1=cb, op=ALU.mult)
                        nc.vector.tensor_tensor(
                            out=t2[:, 0:nr, :], in0=od, in1=sn, op=ALU.mult)
                        nc.vector.tensor_tensor(
                            out=rp[:, 0:nr, 0:64], in0=t1[:, 0:nr, :],
                            in1=t2[:, 0:nr, :], op=ALU.subtract)
                        nc.vector.tensor_tensor(
                            out=t1[:, 0:nr, :], in0=ev, in1=sn, op=ALU.mult)
                        nc.vector.tensor_tensor(
                            out=t2[:, 0:nr, :], in0=od, in1=cb, op=ALU.mult)
                        nc.vector.tensor_tensor(
                            out=rp[:, 0:nr, 64:128], in0=t1[:, 0:nr, :],
                            in1=t2[:, 0:nr, :], op=ALU.add)
                        for h in range(nr):
                            pt = ptp.tile([128, 128], f32r, tag="pt")
                            nc.tensor.transpose(pt, rp[:, h, :], ident)
                            dst = (qT[head_base + h] if kind == "q"
                                   else kT[head_base + h])
                            nc.vector.tensor_copy(
                                out=dst[:, sb * 128:(sb + 1) * 128], in_=pt)

            proj_pass(wkvT[:, :], 2 * GPC * HD, "kv")
            proj_pass(wqT[:, :], HPC * HD, "q", head_base=0)
            s1ctx.__exit__(None, None, None)

            # ------------ Stage 2+3: attention (scoresT) + out-projection ---
            with tc.tile_pool(name="wo2", bufs=1) as wopool, \
                 tc.tile_pool(name="wom2", bufs=2) as womp, \
                 tc.tile_pool(name="pr2", bufs=2) as prpool, \
                 tc.tile_pool(name="att2", bufs=1) as attpool, \
                 tc.tile_pool(name="dn2", bufs=1) as dnpool, \
                 tc.tile_pool(name="o2", bufs=2) as opool, \
                 tc.tile_pool(name="psc", bufs=4, space="PSUM") as pscp, \
                 tc.tile_pool(name="pds", bufs=1, space="PSUM") as pdsp, \
                 tc.tile_pool(name="pav", bufs=2, space="PSUM") as pavp, \
                 tc.tile_pool(name="pou", bufs=1, space="PSUM") as poup:
                for qsb in range(QSB):
                    att = attpool.tile([128, HPC, 512], f32r, tag="att")
                    maxkt = (qsb + 1) * 4 if causal else SB
                    q0g = qsb * 512
                    for g in range(GPC):
                        rr = [dnpool.tile([1, 512], f32r, tag=f"rr{r}",
                                          name=f"rr{r}") for r in range(NREP)]
                        for r in range(NREP):
                            h = g * NREP + r
                            probs = prpool.tile([128, SB, 512], f32r, tag="probs")
                            dsum = pdsp.tile([1, 512], f32, tag="dsum")
                            for t in range(maxkt):
                                # local q start within this superblock
                                ql = max(0, t * 128 - q0g) if causal else 0
                                qw = 512 - ql
                                sc = pscp.tile([128, 512], f32, tag="sc")
                                nc.tensor.matmul(
                                    sc[:, ql:512],
                                    kT[g][:, t * 128:(t + 1) * 128],
                                    qT[h][:, q0g + ql:q0g + 512],
                                    start=True, stop=True)
                                is_diag = causal and t * 128 >= q0g
                                if is_diag:
                                    # add mask pre-scale: exp(SCALE*(sc+msk))
                                    # == exp(SCALE*sc + mask) for the 0/-inf
                                    # mask (underflows to 0 identically)
                                    nc.vector.tensor_tensor(
                                        out=sc[:, ql:ql + 128],
                                        in0=sc[:, ql:ql + 128],
                                        in1=msk, op=ALU.add)
                                nc.scalar.activation(
                                    out=probs[:, t, ql:512],
                                    in_=sc[:, ql:512], func=AF.Exp,
                                    scale=SCALE)
                                nc.tensor.matmul(
                                    dsum[:, ql:512], ones[:, 0:1],
                                    probs[:, t, ql:512],
                                    start=(t == 0), stop=(t == maxkt - 1),
                                    skip_group_check=True)
                                if causal and ql > 0:
                                    # q < k region contributes nothing, but the
                                    # dsum psum slice [0:ql] of t==0 already
                                    # covers it (probs[:,0,0:512] full).
                                    pass
                            # reciprocal row -> R tile via ones-matmul
                            with nc.allow_low_precision(reason="softmax recip"):
                                nc.vector.reciprocal(out=rr[r], in_=dsum)
                            # AV accumulate; normalization happens per group
                            av = pavp.tile([128, 512], f32, tag="av")
                            for t in range(maxkt):
                                ql = max(0, t * 128 - q0g) if causal else 0
                                nc.tensor.matmul(
                                    av[:, ql:512], vsb[g][:, t, :],
                                    probs[:, t, ql:512],
                                    start=(t == 0), stop=(t == maxkt - 1),
                                    skip_group_check=True)
                            nc.vector.tensor_copy(out=att[:, h, :], in_=av)
                        rsb = dnpool.tile([128, 4, 512], f32, tag="rsb")
                        for r in range(NREP):
                            rps = pscp.tile([128, 512], f32, tag="sc")
                            nc.tensor.matmul(
                                rps, ones[0:1, :], rr[r],
                                start=True, stop=True)
                            nc.scalar.copy(out=rsb[:, r, :], in_=rps)
                        for r in range(NREP):
                            h = g * NREP + r
                            nc.vector.tensor_tensor(
                                out=att[:, h, :], in0=att[:, h, :],
                                in1=rsb[:, r, :], op=ALU.mult)
                    # out-projection for this q-superblock
                    for m in range(KT):
                        wom = womp.tile([128, HPC, 128], f32r, tag="wom")
                        nc.sync.dma_start(
                            out=wom, in_=woT3[:, :, m * 128:(m + 1) * 128])
                        wsrc = wom
                        po = poup.tile([128, 512], f32, tag="po")
                        for e in range(HPC):
                            nc.tensor.matmul(
                                po, wsrc[:, e, :], att[:, e, :],
                                start=(e == 0), stop=(e == HPC - 1))
                        ot = opool.tile([128, 512], f32, tag="ot")
                        nc.vector.tensor_copy(out=ot, in_=po)
                        nc.sync.dma_start(
                            out=outT[m * 128:(m + 1) * 128,
                                     qsb * 512:(qsb + 1) * 512],
                            in_=ot)

    nc.compile()
    return nc


def _get_nc(causal: bool):
    if causal not in _compiled:
        _compiled[causal] = _build(causal)
    return _compiled[causal]


def kernel(x, freqs_cis, mask, wq, wk, wv, wo):
    from concourse.bass_utils import run_bass_kernel_spmd

    x = np.asarray(x, dtype=np.float32)
    freqs_cis = np.asarray(freqs_cis, dtype=np.float32)
    mask = np.asarray(mask, dtype=np.float32)
    wq = np.asarray(wq, dtype=np.float32)
    wk = np.asarray(wk, dtype=np.float32)
    wv = np.asarray(wv, dtype=np.float32)
    wo = np.asarray(wo, dtype=np.float32)

    tri = np.tril(np.ones((S, S), dtype=bool))
    causal = bool((mask[tri] == 0.0).all() and (mask[~tri] < -1e30).all())
    if not causal and not (mask == 0.0).all():
        return _numpy_ref(x, freqs_cis, mask, wq, wk, wv, wo)

    nc = _get_nc(causal)

    cos = freqs_cis[:, :, 0]
    sin = freqs_cis[:, :, 1]
    cosS = np.ascontiguousarray(cos.reshape(SB, 128, 64).transpose(1, 0, 2))
    sinS = np.ascontiguousarray(sin.reshape(SB, 128, 64).transpose(1, 0, 2))
    mtile = (np.ascontiguousarray(mask[0:128, 0:128].T) if causal
             else np.zeros((128, 128), dtype=np.float32))
    onest = np.ones((128, 128), dtype=np.float32)

    in_maps = []
    for c in range(8):
        b, i = c // 2, c % 2
        in_maps.append({
            "xT": np.ascontiguousarray(x[b].T),
            "wqT": np.ascontiguousarray(wq[1024 * i:1024 * (i + 1), :].T),
            "wkvT": np.ascontiguousarray(np.concatenate(
                [wk[256 * i:256 * (i + 1), :].T,
                 wv[256 * i:256 * (i + 1), :].T], axis=1)),
            "woT": np.ascontiguousarray(wo[:, 1024 * i:1024 * (i + 1)].T),
            "cosS": cosS, "sinS": sinS, "mtile": mtile, "onest": onest,
        })

    res = run_bass_kernel_spmd(nc, in_maps, core_ids=list(range(8)))
    out = np.empty((B, S, D), dtype=np.float32)
    for b in range(B):
        out[b] = res.results[2 * b]["outT"].T + res.results[2 * b + 1]["outT"].T
    return out


def _numpy_ref(x, freqs_cis, mask, wq, wk, wv, wo):
    xq = (x @ wq.T).reshape(B, S, H, HD)
    xk = (x @ wk.T).reshape(B, S, KV, HD)
    xv = (x @ wv.T).reshape(B, S, KV, HD)

    def rope(xh):
        x2 = xh.reshape(*xh.shape[:-1], HD // 2, 2)
        fc = freqs_cis[None, :, None, :, :]
        real = x2[..., 0] * fc[..., 0] - x2[..., 1] * fc[..., 1]
        imag = x2[..., 0] * fc[..., 1] + x2[..., 1] * fc[..., 0]
        return np.concatenate([real, imag], axis=-1)

    xq, xk = rope(xq), rope(xk)
    q = xq.reshape(B, S, KV, NREP, HD)
    sc = np.einsum('bqgrd,bkgd->bgrqk', q, xk) * SCALE + mask[None, None, None]
    sc = sc - sc.max(axis=-1, keepdims=True)
    p = np.exp(sc)
    p /= p.sum(axis=-1, keepdims=True)
    o = np.einsum('bgrqk,bkgd->bqgrd', p, xv).reshape(B, S, H * HD)
    return (o @ wo.T).astype(np.float32)



# revision 22
# speedup vs baseline: 1.2345x; 1.2345x over previous
"""Trainium2 Bass kernel for nn_Attention (B=4, S=2048, D=2048, H=16, KV=4, HD=128).

Sharding (8 cores): data-parallel over batch (4) x tensor-parallel over
KV-head-group halves (2). Core c handles batch b=c//2 and q-heads
[8*(c%2), 8*(c%2)+8) == kv groups {2*(c%2), 2*(c%2)+1}. Each core produces a
partial output (its heads' contribution through wo); the host sums the two
partials per batch.

All matmul operands are bf16 (PSUM accumulation stays f32): full PE speed at
any tile width, half the DMA bytes, and 1.0-rate PE transposes. Stage 1 is a
single fused pass over x: per s-block, one PSUM accumulation produces
q(8)+k(2)+v(2) head slots; RoPE is applied in [s, hd] layout, then PE
transposes write qT/kT ([hd, s]); v is copied raw. The transposes for s-block
i are emitted after the matmuls of s-block i+1 so the in-order PE never waits
on the DVE rope. Stage 2 computes attention transposed (scoresT[k,q]; kT
stationary, qT moving) so the ACT exp pass doubles as the PSUM->SBUF move.
Softmax denominators use tiny-output matmuls (probs block as stationary, ones
column moving -> [128q, 1] accumulated over k-blocks) instead of re-streaming
probs through a ones-row matmul; the per-head [128,4] reciprocal is
PE-transposed to [4,128] and broadcast to a [128,512] tile via basis-matrix
matmuls, and normalization is fused into the AV PSUM->SBUF move (one DVE
multiply). Each head's denominator tail is deferred by one head so the PE
never waits on it. wo stays resident in SBUF, and the out-projection of
q-superblock i is interleaved between the attention heads of q-superblock
i+1 (two 128-row output blocks per head), which keeps ACT busy with exps
during what used to be a PE-only out-projection phase.
"""
import numpy as np

B, S, D = 4, 2048, 2048
H, KV, HD = 16, 4, 128
NREP = H // KV
SCALE = float(HD) ** -0.5

SB = S // 128          # 16 s-blocks
KT = D // 128          # 16 contraction tiles for projections
QSB = S // 512         # 4 q-superblocks
HPC = 8                # q heads per core
GPC = 2                # kv groups per core
NE = HPC + 2 * GPC     # 12 projection head-slots per core (q0..7, k0, k1, v0, v1)
NR = HPC + GPC         # 10 slots that get RoPE

_compiled = {}


def _build(causal: bool):
    import concourse.bass as bass  # noqa: F401
    import concourse.tile as tile
    from concourse import bacc, mybir
    from concourse.masks import make_identity

    f32 = mybir.dt.float32
    bf16 = mybir.dt.bfloat16
    AF = mybir.ActivationFunctionType
    ALU = mybir.AluOpType

    nc = bacc.Bacc("TRN2")

    # x: [128, SB, KT, 128] with x_dram[p, sb, kt, j] = x[b, sb*128+j, kt*128+p]
    xd = nc.dram_tensor("xd", [128, SB, KT, 128], bf16, kind="ExternalInput")
    # fused qkv weights: wt[p, kt, e] = wcat[e, kt*128+p], e over 12*128
    wd = nc.dram_tensor("wd", [128, KT, NE * 128], bf16, kind="ExternalInput")
    # wo: wod[p, h, d] = wo[d, off + h*128 + p]
    wod = nc.dram_tensor("wod", [128, HPC, D], bf16, kind="ExternalInput")
    cosS = nc.dram_tensor("cosS", [128, SB, 64], f32, kind="ExternalInput")
    sinS = nc.dram_tensor("sinS", [128, SB, 64], f32, kind="ExternalInput")
    mtile = nc.dram_tensor("mtile", [128, 128], f32, kind="ExternalInput")
    onest = nc.dram_tensor("onest", [128, 128], bf16, kind="ExternalInput")
    # basis[k, qb, p] = 1.0 if k == qb else 0 (k, qb in 0..3)
    basist = nc.dram_tensor("basist", [4, 4, 128], bf16, kind="ExternalInput")
    outT = nc.dram_tensor("outT", [D, S], f32, kind="ExternalOutput")

    with tile.TileContext(nc) as tc:
        with tc.tile_pool(name="persist", bufs=1) as persist:
            qT = [persist.tile([128, S], bf16, tag=f"qT{h}", name=f"qT{h}") for h in range(HPC)]
            kT = [persist.tile([128, S], bf16, tag=f"kTg{g}", name=f"kTg{g}") for g in range(GPC)]
            vsb = [persist.tile([128, SB, 128], bf16, tag=f"v{g}", name=f"v{g}") for g in range(GPC)]
            msk = persist.tile([128, 128], f32, tag="msk")
            ones = persist.tile([128, 128], bf16, tag="ones")
            zer4 = persist.tile([128, 4], bf16, tag="zer4")
            basis = persist.tile([4, 4, 128], bf16, tag="basis")
            wo_sb = persist.tile([128, HPC, D], bf16, tag="wo")
            ident = persist.tile([128, 128], bf16, tag="ident")
            cos_t = persist.tile([128, SB, 64], f32, tag="cos")
            sin_t = persist.tile([128, SB, 64], f32, tag="sin")
            psb = persist.tile([128, NR, 128], f32, tag="psb")
            rp15 = persist.tile([128, NR, 128], bf16, tag="rp15")

            # ------------ Stage 1: fused projections + RoPE + transposes ----
            s1ctx = tc.tile_pool(name="s1const", bufs=1)
            s1const = s1ctx.__enter__()
            ident_f = s1const.tile([128, 128], f32, tag="identf")
            make_identity(nc, ident_f)
            nc.vector.tensor_copy(out=ident, in_=ident_f)
            nc.vector.tensor_tensor(
                out=zer4, in0=ident[:, 0:4], in1=ident[:, 0:4],
                op=ALU.subtract)
            # preload the ACT exp table so the first real exp (stage 2)
            # doesn't pay the table-load latency
            warm = s1const.tile([1, 8], f32, tag="warm")
            nc.scalar.activation(
                out=warm, in_=ident_f[0:1, 0:8], func=AF.Exp, scale=1.0)

            EW = NE * 128  # 1536
            with tc.tile_pool(name="w1", bufs=1) as wpool, \
                 tc.tile_pool(name="xs1", bufs=3) as xpool, \
                 tc.tile_pool(name="rs1", bufs=2) as rpool, \
                 tc.tile_pool(name="pq1", bufs=2, space="PSUM") as pqp, \
                 tc.tile_pool(name="pt1", bufs=2, space="PSUM") as ptp:
                wt = wpool.tile([128, KT, EW], bf16, tag="wt")

                def make_trans(rp):
                    def emit():
                        for h in range(NR):
                            pt = ptp.tile([128, 128], bf16, tag="pt")
                            nc.tensor.transpose(pt, rp[:, h, :], ident)
                            dst = qT[h] if h < HPC else kT[h - HPC]
                            nc.vector.tensor_copy(
                                out=dst[:, sb * 128:(sb + 1) * 128], in_=pt)
                    # bind loop variable
                    sb = emit_sb
                    return emit

                pending = None
                for sb in range(SB):
                    xs = xpool.tile([128, KT, 128], bf16, tag="xs")
                    nc.sync.dma_start(out=xs, in_=xd[:, sb, :, :])
                    if sb == 0:
                        # weight chunks issued after the first x tile so the
                        # first matmuls can start as soon as possible; cos/sin
                        # slip in early (rope of s-block 0 needs them), the
                        # stage-2 constants go last
                        for kt in range(2):
                            nc.sync.dma_start(out=wt[:, kt, :], in_=wd[:, kt, :])
                        nc.sync.dma_start(out=cos_t, in_=cosS[:, :, :])
                        nc.sync.dma_start(out=sin_t, in_=sinS[:, :, :])
                        for kt in range(2, KT):
                            nc.sync.dma_start(out=wt[:, kt, :], in_=wd[:, kt, :])
                    if sb == 1:
                        nc.sync.dma_start(out=msk, in_=mtile[:, :])
                        nc.sync.dma_start(out=ones, in_=onest[:, :])
                        nc.sync.dma_start(out=basis, in_=basist[:, :, :])
                    if sb == SB - 1:
                        # wo arrives during the tail of stage 1
                        nc.sync.dma_start(out=wo_sb, in_=wod[:, :, :])
                    ps = pqp.tile([128, NE, 128], f32, tag="ps")
                    ps2 = ps.rearrange("p h d -> p (h d)")
                    for kt in range(KT):
                        for n0 in range(0, EW, 512):
                            nc.tensor.matmul(
                                ps2[:, n0:n0 + 512], xs[:, kt, :],
                                wt[:, kt, n0:n0 + 512],
                                start=(kt == 0), stop=(kt == KT - 1))
                    for g in range(GPC):
                        nc.scalar.copy(
                            out=vsb[g][:, sb, :], in_=ps[:, HPC + GPC + g, :])
                    # transposes of the previous s-block (rope long done)
                    if pending is not None:
                        pending()
                    if sb == SB - 1:
                        # copy q/k slots out of PSUM so the stage-1 PSUM pools
                        # release early (stage 2 reuses the banks)
                        nc.scalar.copy(out=psb, in_=ps[:, 0:NR, :])
                        src = psb
                    else:
                        src = ps
                    # RoPE on the 10 q/k head slots (last s-block's result
                    # goes to a persistent tile; its transposes are emitted
                    # inside stage 2 so the PE never waits on the final rope)
                    if sb == SB - 1:
                        rp = rp15
                    else:
                        rp = rpool.tile([128, NR, 128], bf16, tag="rope")
                    ev = src[:, 0:NR, 0:128:2]
                    od = src[:, 0:NR, 1:128:2]
                    cb = cos_t[:, None, sb, :].broadcast_to([128, NR, 64])
                    sn = sin_t[:, None, sb, :].broadcast_to([128, NR, 64])
                    t1 = rpool.tile([128, NR, 64], f32, tag="t1")
                    t2 = rpool.tile([128, NR, 64], f32, tag="t2")
                    nc.vector.tensor_tensor(out=t1, in0=ev, in1=cb, op=ALU.mult)
                    nc.vector.tensor_tensor(out=t2, in0=od, in1=sn, op=ALU.mult)
                    nc.vector.tensor_tensor(
                        out=rp[:, :, 0:64], in0=t1, in1=t2, op=ALU.subtract)
                    nc.vector.tensor_tensor(out=t1, in0=ev, in1=sn, op=ALU.mult)
                    nc.vector.tensor_tensor(out=t2, in0=od, in1=cb, op=ALU.mult)
                    nc.vector.tensor_tensor(
                        out=rp[:, :, 64:128], in0=t1, in1=t2, op=ALU.add)
                    emit_sb = sb
                    if sb < SB - 1:
                        pending = make_trans(rp)
            s1ctx.__exit__(None, None, None)

            # ------------ Stage 2: attention (scoresT) + out-projection -----
            with tc.tile_pool(name="pr2", bufs=2) as prpool, \
                 tc.tile_pool(name="att2", bufs=2) as attpool, \
                 tc.tile_pool(name="dn2", bufs=2) as dnpool, \
                 tc.tile_pool(name="o2", bufs=2) as opool, \
                 tc.tile_pool(name="psc", bufs=3, space="PSUM") as pscp, \
                 tc.tile_pool(name="pav", bufs=2, space="PSUM") as pavp, \
                 tc.tile_pool(name="pds", bufs=1, space="PSUM") as pdsp, \
                 tc.tile_pool(name="scr", bufs=2, space="PSUM") as scrp:

                def make_trans15(j_):
                    def emit():
                        pt = scrp.tile([128, 128], bf16, tag="scr", name="pt15")
                        nc.tensor.transpose(pt, rp15[:, j_, :], ident)
                        dst = qT[j_] if j_ < HPC else kT[j_ - HPC]
                        nc.vector.tensor_copy(
                            out=dst[:, (SB - 1) * 128:SB * 128], in_=pt)
                    return emit

                trans15 = [make_trans15(j) for j in range(NR)]

                def make_tail(h_, av_, rrh_, att_):
                    # part a: transpose the reciprocal row; copy it to SBUF on
                    # DVE (its backlog is far shorter than ACT's exp queue)
                    rrow = [None]

                    def emit_a():
                        trp = scrp.tile([4, 128], bf16, tag="scr")
                        nc.tensor.transpose(trp, rrh_, ident)
                        rrow[0] = dnpool.tile(
                            [4, 128], bf16, tag="rrow", name="rrow")
                        nc.vector.tensor_copy(out=rrow[0], in_=trp)

                    def emit_b():
                        rps = scrp.tile([128, 512], f32, tag="scr")
                        for qb in range(4):
                            nc.tensor.matmul(
                                rps[:, qb * 128:(qb + 1) * 128],
                                basis[:, qb, :], rrow[0],
                                start=True, stop=True)
                        # DVE may read only ONE operand from PSUM: stage the
                        # broadcast tile to SBUF (bf16), then multiply it into
                        # the AV PSUM on the way to the att tile
                        rsb = dnpool.tile([128, 512], bf16, tag="rsb")
                        nc.vector.tensor_copy(out=rsb, in_=rps)
                        nc.vector.tensor_tensor(
                            out=att_[:, h_, :], in0=av_, in1=rsb,
                            op=ALU.mult)
                    return emit_a, emit_b

                def make_po(att_, qsb_, m_):
                    def emit():
                        po = scrp.tile([128, 512], f32, tag="scr")
                        for e in range(HPC):
                            nc.tensor.matmul(
                                po, wo_sb[:, e, m_ * 128:(m_ + 1) * 128],
                                att_[:, e, :],
                                start=(e == 0), stop=(e == HPC - 1))
                        ot = opool.tile([128, 512], f32, tag="ot")
                        nc.vector.tensor_copy(out=ot, in_=po)
                        nc.sync.dma_start(
                            out=outT[m_ * 128:(m_ + 1) * 128,
                                     qsb_ * 512:(qsb_ + 1) * 512],
                            in_=ot)
                    return emit

                po_queue = []
                tail_prev = None
                for qsb in range(QSB):
                    att = attpool.tile([128, HPC, 512], bf16, tag="att")
                    maxkt = (qsb + 1) * 4 if causal else SB
                    q0g = qsb * 512
                    for g in range(GPC):
                        for r in range(NREP):
                            h = g * NREP + r
                            probs = prpool.tile([128, SB, 512], bf16, tag="probs")
                            dsT = pdsp.tile([128, 4], f32, tag="dsT")
                            av = pavp.tile([128, 512], f32, tag="av")
                            # PSUM accumulation groups must be exclusive per
                            # bank on HW: zero-init the 4-column denominator
                            # tile and accumulate with start=False throughout
                            nc.vector.tensor_copy(out=dsT, in_=zer4)
                            for t in range(maxkt):
                                ql = max(0, t * 128 - q0g) if causal else 0
                                sc = pscp.tile([128, 512], f32, tag="sc")
                                nc.tensor.matmul(
                                    sc[:, ql:512],
                                    kT[g][:, t * 128:(t + 1) * 128],
                                    qT[h][:, q0g + ql:q0g + 512],
                                    start=True, stop=True)
                                is_diag = causal and t * 128 >= q0g
                                if is_diag:
                                    # add mask pre-scale: exp(SCALE*(sc+msk))
                                    # == exp(SCALE*sc + mask) for the 0/-inf
                                    # mask (underflows to 0 identically)
                                    nc.vector.tensor_tensor(
                                        out=sc[:, ql:ql + 128],
                                        in0=sc[:, ql:ql + 128],
                                        in1=msk, op=ALU.add)
                                nc.scalar.activation(
                                    out=probs[:, t, ql:512],
                                    in_=sc[:, ql:512], func=AF.Exp,
                                    scale=SCALE)
                                nc.tensor.matmul(
                                    av[:, ql:512], vsb[g][:, t, :],
                                    probs[:, t, ql:512],
                                    start=(t == 0), stop=(t == maxkt - 1),
                                    skip_group_check=True)
                                # per-q-block denominator partials: tiny-output
                                # matmuls (probs block stationary, ones moving)
                                qb0 = max(0, t - qsb * 4) if causal else 0
                                for qb in range(qb0, 4):
                                    tlast = qsb * 4 + qb if causal else maxkt - 1
                                    nc.tensor.matmul(
                                        dsT[:, qb:qb + 1],
                                        probs[:, t, qb * 128:(qb + 1) * 128],
                                        ones[:, 0:1],
                                        start=False, stop=(t == tlast),
                                        skip_group_check=True)
                            rrh = dnpool.tile([128, 4], bf16, tag="rrh")
                            with nc.allow_low_precision(reason="softmax recip"):
                                nc.vector.reciprocal(out=rrh, in_=dsT)
                            # deferred work: previous head's denominator tail
                            # interleaved with two out-proj blocks of the
                            # previous qsb (the po blocks cover the Pool-copy
                            # latency and let ACT drain its exp backlog)
                            if tail_prev is not None:
                                tail_prev[0]()
                            if po_queue:
                                po_queue.pop(0)()
                            if tail_prev is not None:
                                tail_prev[1]()
                            if po_queue:
                                po_queue.pop(0)()
                            if qsb == 0 and h >= 4:
                                for _ in range(3):
                                    if trans15:
                                        trans15.pop(0)()
                            tail_prev = make_tail(h, av, rrh, att)
                    tail_prev[0]()
                    tail_prev[1]()
                    tail_prev = None
                    po_queue = [make_po(att, qsb, m) for m in range(KT)]
                    if qsb == QSB - 1:
                        while po_queue:
                            po_queue.pop(0)()

    nc.compile()
    return nc


def _get_nc(causal: bool):
    if causal not in _compiled:
        _compiled[causal] = _build(causal)
    return _compiled[causal]


def kernel(x, freqs_cis, mask, wq, wk, wv, wo):
    import ml_dtypes
    from concourse.bass_utils import run_bass_kernel_spmd

    bf = ml_dtypes.bfloat16
    x = np.asarray(x, dtype=np.float32)
    freqs_cis = np.asarray(freqs_cis, dtype=np.float32)
    mask = np.asarray(mask, dtype=np.float32)
    wq = np.asarray(wq, dtype=np.float32)
    wk = np.asarray(wk, dtype=np.float32)
    wv = np.asarray(wv, dtype=np.float32)
    wo = np.asarray(wo, dtype=np.float32)

    tri = np.tril(np.ones((S, S), dtype=bool))
    causal = bool((mask[tri] == 0.0).all() and (mask[~tri] < -1e30).all())
    if not causal and not (mask == 0.0).all():
        return _numpy_ref(x, freqs_cis, mask, wq, wk, wv, wo)

    nc = _get_nc(causal)

    cos = freqs_cis[:, :, 0]
    sin = freqs_cis[:, :, 1]
    cosS = np.ascontiguousarray(cos.reshape(SB, 128, 64).transpose(1, 0, 2))
    sinS = np.ascontiguousarray(sin.reshape(SB, 128, 64).transpose(1, 0, 2))
    mtile = (np.ascontiguousarray(mask[0:128, 0:128].T) if causal
             else np.zeros((128, 128), dtype=np.float32))
    onest = np.ones((128, 128), dtype=bf)
    basist = np.ascontiguousarray(
        np.broadcast_to(np.eye(4, dtype=bf)[:, :, None], (4, 4, 128)))

    in_maps = []
    for c in range(8):
        b, i = c // 2, c % 2
        # x[b]: [S, D] -> [128 p, SB, KT, 128 j]
        xd = np.ascontiguousarray(
            x[b].reshape(SB, 128, KT, 128).transpose(3, 0, 2, 1).astype(bf))
        wcat = np.concatenate(
            [wq[1024 * i:1024 * (i + 1), :],
             wk[256 * i:256 * (i + 1), :],
             wv[256 * i:256 * (i + 1), :]], axis=0)  # [1536, D]
        wd = np.ascontiguousarray(
            wcat.T.reshape(KT, 128, NE * 128).transpose(1, 0, 2).astype(bf))
        wod = np.ascontiguousarray(
            wo[:, 1024 * i:1024 * (i + 1)].T.reshape(HPC, 128, D)
            .transpose(1, 0, 2).astype(bf))
        in_maps.append({
            "xd": xd, "wd": wd, "wod": wod,
            "cosS": cosS, "sinS": sinS, "mtile": mtile, "onest": onest,
            "basist": basist,
        })

    res = run_bass_kernel_spmd(nc, in_maps, core_ids=list(range(8)))
    out = np.empty((B, S, D), dtype=np.float32)
    for b in range(B):
        out[b] = res.results[2 * b]["outT"].T + res.results[2 * b + 1]["outT"].T
    return out


def _numpy_ref(x, freqs_cis, mask, wq, wk, wv, wo):
    xq = (x @ wq.T).reshape(B, S, H, HD)
    xk = (x @ wk.T).reshape(B, S, KV, HD)
    xv = (x @ wv.T).reshape(B, S, KV, HD)

    def rope(xh):
        x2 = xh.reshape(*xh.shape[:-1], HD // 2, 2)
        fc = freqs_cis[None, :, None, :, :]
        real = x2[..., 0] * fc[..., 0] - x2[..., 1] * fc[..., 1]
        imag = x2[..., 0] * fc[..., 1] + x2[..., 1] * fc[..., 0]
        return np.concatenate([real, imag], axis=-1)

    xq, xk = rope(xq), rope(xk)
    q = xq.reshape(B, S, KV, NREP, HD)
    sc = np.einsum('bqgrd,bkgd->bgrqk', q, xk) * SCALE + mask[None, None, None]
    sc = sc - sc.max(axis=-1, keepdims=True)
    p = np.exp(sc)
    p /= p.sum(axis=-1, keepdims=True)
    o = np.einsum('bgrqk,bkgd->bqgrd', p, xv).reshape(B, S, H * HD)
    return (o @ wo.T).astype(np.float32)


# revision 36
# speedup vs baseline: 1.2517x; 1.0139x over previous
"""Trainium2 Bass kernel for nn_Attention (B=4, S=2048, D=2048, H=16, KV=4, HD=128).

Sharding (8 cores): data-parallel over batch (4) x tensor-parallel over
KV-head-group halves (2). Core c handles batch b=c//2 and q-heads
[8*(c%2), 8*(c%2)+8) == kv groups {2*(c%2), 2*(c%2)+1}. Each core produces a
partial output (its heads' contribution through wo); the host sums the two
partials per batch.

All matmul operands are bf16 (PSUM accumulation stays f32): full PE speed at
any tile width, half the DMA bytes, and 1.0-rate PE transposes. Stage 1 is a
single fused pass over x: per s-block, one PSUM accumulation produces
q(8)+k(2)+v(2) head slots; RoPE is applied in [s, hd] layout, then PE
transposes write qT/kT ([hd, s]); v is copied raw. The transposes for s-block
i are emitted after the matmuls of s-block i+1 so the in-order PE never waits
on the DVE rope. Stage 2 computes attention transposed (scoresT[k,q]; kT
stationary, qT moving) so the ACT exp pass doubles as the PSUM->SBUF move.
Softmax denominators use tiny-output matmuls (probs block as stationary, ones
column moving -> [128q, 1] accumulated over k-blocks) instead of re-streaming
probs through a ones-row matmul; the per-head [128,4] reciprocal is
PE-transposed to [4,128] and broadcast to a [128,512] tile via basis-matrix
matmuls, and normalization is fused into the AV PSUM->SBUF move (one DVE
multiply). Each head's denominator tail is deferred by one head so the PE
never waits on it. wo stays resident in SBUF, and the out-projection of
q-superblock i is interleaved between the attention heads of q-superblock
i+1 (two 128-row output blocks per head), which keeps ACT busy with exps
during what used to be a PE-only out-projection phase.
"""
import numpy as np

B, S, D = 4, 2048, 2048
H, KV, HD = 16, 4, 128
NREP = H // KV
SCALE = float(HD) ** -0.5

SB = S // 128          # 16 s-blocks
KT = D // 128          # 16 contraction tiles for projections
QSB = S // 512         # 4 q-superblocks
HPC = 8                # q heads per core
GPC = 2                # kv groups per core
NE = HPC + 2 * GPC     # 12 projection head-slots per core (q0..7, k0, k1, v0, v1)
NR = HPC + GPC         # 10 slots that get RoPE

_compiled = {}


def _build(causal: bool):
    import concourse.bass as bass  # noqa: F401
    import concourse.tile as tile
    from concourse import bacc, mybir
    from concourse.masks import make_identity

    f32 = mybir.dt.float32
    bf16 = mybir.dt.bfloat16
    AF = mybir.ActivationFunctionType
    ALU = mybir.AluOpType

    nc = bacc.Bacc("TRN2")

    # x: [128, SB, KT, 128] with x_dram[p, sb, kt, j] = x[b, sb*128+j, kt*128+p]
    xd = nc.dram_tensor("xd", [128, SB, KT, 128], bf16, kind="ExternalInput")
    # fused qkv weights: wt[p, kt, e] = wcat[e, kt*128+p], e over 12*128
    wd = nc.dram_tensor("wd", [128, KT, NE * 128], bf16, kind="ExternalInput")
    # wo: wod[p, h, d] = wo[d, off + h*128 + p]
    wod = nc.dram_tensor("wod", [128, HPC, D], bf16, kind="ExternalInput")
    cosS = nc.dram_tensor("cosS", [128, SB, 64], f32, kind="ExternalInput")
    sinS = nc.dram_tensor("sinS", [128, SB, 64], f32, kind="ExternalInput")
    mtile = nc.dram_tensor("mtile", [128, 128], f32, kind="ExternalInput")
    onest = nc.dram_tensor("onest", [128, 128], bf16, kind="ExternalInput")
    # basis[k, qb, p] = 1.0 if k == qb else 0 (k, qb in 0..3)
    basist = nc.dram_tensor("basist", [4, 4, 128], bf16, kind="ExternalInput")
    outT = nc.dram_tensor("outT", [D, S], f32, kind="ExternalOutput")

    with tile.TileContext(nc) as tc:
        with tc.tile_pool(name="persist", bufs=1) as persist:
            qT = [persist.tile([128, S], bf16, tag=f"qT{h}", name=f"qT{h}") for h in range(HPC)]
            kT = [persist.tile([128, S], bf16, tag=f"kTg{g}", name=f"kTg{g}") for g in range(GPC)]
            vsb = [persist.tile([128, SB, 128], bf16, tag=f"v{g}", name=f"v{g}") for g in range(GPC)]
            msk = persist.tile([128, 128], f32, tag="msk")
            ones = persist.tile([128, 128], bf16, tag="ones")
            zer4 = persist.tile([128, 4], bf16, tag="zer4")
            basis = persist.tile([4, 4, 128], bf16, tag="basis")
            wo_sb = persist.tile([128, HPC, D], bf16, tag="wo")
            ident = persist.tile([128, 128], bf16, tag="ident")
            cos_t = persist.tile([128, SB, 64], f32, tag="cos")
            sin_t = persist.tile([128, SB, 64], f32, tag="sin")
            psb = persist.tile([128, NR, 128], f32, tag="psb")
            psb0 = persist.tile([128, NR, 128], f32, tag="psb0")
            psb1 = persist.tile([128, NR, 128], f32, tag="psb1")
            # late s-blocks' rope output lives in persistent tiles; their
            # transposes are emitted inside stage 2 (fills ACT-bound gaps)
            NLATE = 3
            rpLate = [persist.tile([128, NR, 128], bf16, tag=f"rpL{i}",
                                   name=f"rpL{i}") for i in range(NLATE)]

            # ------------ Stage 1: fused projections + RoPE + transposes ----
            s1ctx = tc.tile_pool(name="s1const", bufs=1)
            s1const = s1ctx.__enter__()
            ident_f = s1const.tile([128, 128], f32, tag="identf")
            make_identity(nc, ident_f)
            nc.vector.tensor_copy(out=ident, in_=ident_f)
            nc.vector.tensor_tensor(
                out=zer4, in0=ident[:, 0:4], in1=ident[:, 0:4],
                op=ALU.subtract)
            # preload the ACT exp table so the first real exp (stage 2)
            # doesn't pay the table-load latency
            warm = s1const.tile([1, 8], f32, tag="warm")
            nc.scalar.activation(
                out=warm, in_=ident_f[0:1, 0:8], func=AF.Exp, scale=1.0)

            EW = NE * 128  # 1536
            with tc.tile_pool(name="w1", bufs=1) as wpool, \
                 tc.tile_pool(name="xs1", bufs=3) as xpool, \
                 tc.tile_pool(name="rs1", bufs=3) as rpool, \
                 tc.tile_pool(name="pq1", bufs=2, space="PSUM") as pqp, \
                 tc.tile_pool(name="pt1", bufs=2, space="PSUM") as ptp:
                wt = wpool.tile([128, KT, EW], bf16, tag="wt")

                def make_trans(rp, sb_):
                    def emit():
                        for h in range(NR):
                            pt = ptp.tile([128, 128], bf16, tag="pt")
                            nc.tensor.transpose(pt, rp[:, h, :], ident)
                            dst = qT[h] if h < HPC else kT[h - HPC]
                            nc.vector.tensor_copy(
                                out=dst[:, sb_ * 128:(sb_ + 1) * 128], in_=pt)
                    return emit

                def emit_rope(ps, sb):
                    for g in range(GPC):
                        nc.scalar.copy(
                            out=vsb[g][:, sb, :], in_=ps[:, HPC + GPC + g, :])
                    if sb <= 1 or sb == SB - 1:
                        # copy q/k slots out of PSUM so the PSUM buffer frees
                        # early (sb0/sb1: the fused prefix holds both ps
                        # buffers; sb15: stage 2 reuses the banks)
                        dst_ps = (psb0, psb1, psb)[min(sb, 2)]
                        nc.scalar.copy(out=dst_ps, in_=ps[:, 0:NR, :])
                        src = dst_ps
                    else:
                        src = ps
                    if sb >= SB - NLATE:
                        rp = rpLate[sb - (SB - NLATE)]
                    else:
                        rp = rpool.tile([128, NR, 128], bf16, tag="rope",
                                        name="rope")
                    ev = src[:, 0:NR, 0:128:2]
                    od = src[:, 0:NR, 1:128:2]
                    cb = cos_t[:, None, sb, :].broadcast_to([128, NR, 64])
                    sn = sin_t[:, None, sb, :].broadcast_to([128, NR, 64])
                    t1 = rpool.tile([128, NR, 64], f32, tag="t1", name="t1")
                    t2 = rpool.tile([128, NR, 64], f32, tag="t2", name="t2")
                    nc.vector.tensor_tensor(out=t1, in0=ev, in1=cb, op=ALU.mult)
                    nc.vector.tensor_tensor(out=t2, in0=od, in1=sn, op=ALU.mult)
                    nc.vector.tensor_tensor(
                        out=rp[:, :, 0:64], in0=t1, in1=t2, op=ALU.subtract)
                    nc.vector.tensor_tensor(out=t1, in0=ev, in1=sn, op=ALU.mult)
                    nc.vector.tensor_tensor(out=t2, in0=od, in1=cb, op=ALU.mult)
                    nc.vector.tensor_tensor(
                        out=rp[:, :, 64:128], in0=t1, in1=t2, op=ALU.add)
                    return rp

                def emit_mm(ps2, xs, kt):
                    for n0 in range(0, EW, 512):
                        nc.tensor.matmul(
                            ps2[:, n0:n0 + 512], xs[:, kt, :],
                            wt[:, kt, n0:n0 + 512],
                            start=(kt == 0), stop=(kt == KT - 1))

                pending = []
                # fused prefix: sb0+sb1 interleaved per weight chunk so the
                # PE has two s-blocks of work while the wt DMA streams in
                xs0 = xpool.tile([128, KT, 128], bf16, tag="xs", name="xs")
                xs1 = xpool.tile([128, KT, 128], bf16, tag="xs", name="xs")
                nc.sync.dma_start(out=xs0[:, 0:4, :], in_=xd[:, 0, 0:4, :])
                nc.sync.dma_start(out=xs1[:, 0:4, :], in_=xd[:, 1, 0:4, :])
                nc.sync.dma_start(out=wt[:, 0, :], in_=wd[:, 0, :])
                nc.sync.dma_start(out=wt[:, 1, :], in_=wd[:, 1, :])
                nc.sync.dma_start(out=wt[:, 2, :], in_=wd[:, 2, :])
                nc.sync.dma_start(out=xs0[:, 4:KT, :], in_=xd[:, 0, 4:KT, :])
                nc.sync.dma_start(out=xs1[:, 4:KT, :], in_=xd[:, 1, 4:KT, :])
                for kt in range(3, 6):
                    nc.sync.dma_start(out=wt[:, kt, :], in_=wd[:, kt, :])
                nc.sync.dma_start(out=cos_t, in_=cosS[:, :, :])
                nc.sync.dma_start(out=sin_t, in_=sinS[:, :, :])
                for kt in range(6, KT):
                    nc.sync.dma_start(out=wt[:, kt, :], in_=wd[:, kt, :])
                nc.sync.dma_start(out=msk, in_=mtile[:, :])
                nc.sync.dma_start(out=ones, in_=onest[:, :])
                nc.sync.dma_start(out=basis, in_=basist[:, :, :])
                ps0 = pqp.tile([128, NE, 128], f32, tag="ps", name="ps")
                ps1 = pqp.tile([128, NE, 128], f32, tag="ps", name="ps")
                ps0f = ps0.rearrange("p h d -> p (h d)")
                ps1f = ps1.rearrange("p h d -> p (h d)")
                for kt in range(KT):
                    emit_mm(ps0f, xs0, kt)
                    emit_mm(ps1f, xs1, kt)
                rp0 = emit_rope(ps0, 0)
                rp1 = emit_rope(ps1, 1)
                pending.append(make_trans(rp0, 0))
                pending.append(make_trans(rp1, 1))

                for sb in range(2, SB):
                    xs = xpool.tile([128, KT, 128], bf16, tag="xs", name="xs")
                    nc.sync.dma_start(out=xs, in_=xd[:, sb, :, :])
                    if sb == SB - 1:
                        # wo arrives during the tail of stage 1
                        nc.sync.dma_start(out=wo_sb, in_=wod[:, :, :])
                    ps = pqp.tile([128, NE, 128], f32, tag="ps", name="ps")
                    ps2 = ps.rearrange("p h d -> p (h d)")
                    for kt in range(KT):
                        emit_mm(ps2, xs, kt)
                    # transposes of an earlier s-block (rope long done)
                    if pending:
                        pending.pop(0)()
                    rp = emit_rope(ps, sb)
                    if sb < SB - NLATE:
                        pending.append(make_trans(rp, sb))
                while pending:
                    pending.pop(0)()
            s1ctx.__exit__(None, None, None)

            # ------------ Stage 2: attention (scoresT) + out-projection -----
            with tc.tile_pool(name="pr2", bufs=2) as prpool, \
                 tc.tile_pool(name="att2", bufs=2) as attpool, \
                 tc.tile_pool(name="dn2", bufs=2) as dnpool, \
                 tc.tile_pool(name="o2", bufs=2) as opool, \
                 tc.tile_pool(name="psc", bufs=3, space="PSUM") as pscp, \
                 tc.tile_pool(name="pav", bufs=2, space="PSUM") as pavp, \
                 tc.tile_pool(name="pds", bufs=1, space="PSUM") as pdsp, \
                 tc.tile_pool(name="scr", bufs=2, space="PSUM") as scrp:

                def make_transL(i_, j_):
                    def emit():
                        pt = scrp.tile([128, 128], bf16, tag="scr", name="ptL")
                        nc.tensor.transpose(pt, rpLate[i_][:, j_, :], ident)
                        dst = qT[j_] if j_ < HPC else kT[j_ - HPC]
                        sbL = SB - NLATE + i_
                        nc.vector.tensor_copy(
                            out=dst[:, sbL * 128:(sbL + 1) * 128], in_=pt)
                    return emit

                transL = [make_transL(i, j)
                          for i in range(NLATE) for j in range(NR)]

                def make_tail(h_, av_, rrh_, att_, qsb_):
                    # part a: transpose the reciprocal row; copy it to SBUF on
                    # DVE (its backlog is far shorter than ACT's exp queue)
                    rrow = [None]

                    def emit_a():
                        trp = scrp.tile([4, 128], bf16, tag="scr")
                        nc.tensor.transpose(trp, rrh_, ident)
                        rrow[0] = dnpool.tile(
                            [4, 128], bf16, tag="rrow", name="rrow")
                        nc.vector.tensor_copy(out=rrow[0], in_=trp)

                    def emit_b():
                        rps = scrp.tile([128, 512], f32, tag="scr")
                        for qb in range(4):
                            nc.tensor.matmul(
                                rps[:, qb * 128:(qb + 1) * 128],
                                basis[:, qb, :], rrow[0],
                                start=True, stop=True)
                        # DVE may read only ONE operand from PSUM: stage the
                        # broadcast tile to SBUF (bf16), then multiply it into
                        # the AV PSUM on the way to the att tile. In qsb0 the
                        # DVE is the regional bottleneck, so stage on ACT there
                        rsb = dnpool.tile([128, 512], bf16, tag="rsb")
                        if qsb_ == 0:
                            nc.scalar.copy(out=rsb, in_=rps)
                        else:
                            nc.vector.tensor_copy(out=rsb, in_=rps)
                        nc.vector.tensor_tensor(
                            out=att_[:, h_, :], in0=av_, in1=rsb,
                            op=ALU.mult)
                    return emit_a, emit_b

                def make_po(att_, qsb_, m_):
                    def emit():
                        po = scrp.tile([128, 512], f32, tag="scr")
                        for e in range(HPC):
                            nc.tensor.matmul(
                                po, wo_sb[:, e, m_ * 128:(m_ + 1) * 128],
                                att_[:, e, :],
                                start=(e == 0), stop=(e == HPC - 1))
                        ot = opool.tile([128, 512], f32, tag="ot")
                        nc.vector.tensor_copy(out=ot, in_=po)
                        nc.sync.dma_start(
                            out=outT[m_ * 128:(m_ + 1) * 128,
                                     qsb_ * 512:(qsb_ + 1) * 512],
                            in_=ot)
                    return emit

                po_queue = []
                tail_prev = None
                for qsb in range(QSB):
                    att = attpool.tile([128, HPC, 512], bf16, tag="att")
                    maxkt = (qsb + 1) * 4 if causal else SB
                    q0g = qsb * 512
                    for g in range(GPC):
                        for r in range(NREP):
                            h = g * NREP + r
                            probs = prpool.tile([128, SB, 512], bf16, tag="probs")
                            dsT = pdsp.tile([128, 4], f32, tag="dsT")
                            av = pavp.tile([128, 512], f32, tag="av")
                            # PSUM accumulation groups must be exclusive per
                            # bank on HW: zero-init the 4-column denominator
                            # tile and accumulate with start=False throughout
                            nc.vector.tensor_copy(out=dsT, in_=zer4)
                            for t in range(maxkt):
                                ql = max(0, t * 128 - q0g) if causal else 0
                                sc = pscp.tile([128, 512], f32, tag="sc")
                                nc.tensor.matmul(
                                    sc[:, ql:512],
                                    kT[g][:, t * 128:(t + 1) * 128],
                                    qT[h][:, q0g + ql:q0g + 512],
                                    start=True, stop=True)
                                is_diag = causal and t * 128 >= q0g
                                if is_diag:
                                    # add mask pre-scale: exp(SCALE*(sc+msk))
                                    # == exp(SCALE*sc + mask) for the 0/-inf
                                    # mask (underflows to 0 identically)
                                    nc.vector.tensor_tensor(
                                        out=sc[:, ql:ql + 128],
                                        in0=sc[:, ql:ql + 128],
                                        in1=msk, op=ALU.add)
                                nc.scalar.activation(
                                    out=probs[:, t, ql:512],
                                    in_=sc[:, ql:512], func=AF.Exp,
                                    scale=SCALE)
                                nc.tensor.matmul(
                                    av[:, ql:512], vsb[g][:, t, :],
                                    probs[:, t, ql:512],
                                    start=(t == 0), stop=(t == maxkt - 1),
                                    skip_group_check=True)
                                # per-q-block denominator partials: tiny-output
                                # matmuls (probs block stationary, ones moving)
                                qb0 = max(0, t - qsb * 4) if causal else 0
                                for qb in range(qb0, 4):
                                    tlast = qsb * 4 + qb if causal else maxkt - 1
                                    nc.tensor.matmul(
                                        dsT[:, qb:qb + 1],
                                        probs[:, t, qb * 128:(qb + 1) * 128],
                                        ones[:, 0:1],
                                        start=False, stop=(t == tlast),
                                        skip_group_check=True)
                            rrh = dnpool.tile([128, 4], bf16, tag="rrh")
                            with nc.allow_low_precision(reason="softmax recip"):
                                nc.vector.reciprocal(out=rrh, in_=dsT)
                            # deferred work: previous head's denominator tail
                            # interleaved with two out-proj blocks of the
                            # previous qsb (the po blocks cover the Pool-copy
                            # latency and let ACT drain its exp backlog)
                            if tail_prev is not None:
                                tail_prev[0]()
                            if po_queue:
                                po_queue.pop(0)()
                            if tail_prev is not None:
                                tail_prev[1]()
                            if po_queue:
                                po_queue.pop(0)()
                            if qsb <= 1:
                                for _ in range(2):
                                    if transL:
                                        transL.pop(0)()
                            tail_prev = make_tail(h, av, rrh, att, qsb)
                    tail_prev[0]()
                    tail_prev[1]()
                    tail_prev = None
                    po_queue = [make_po(att, qsb, m) for m in range(KT)]
                    if qsb == QSB - 1:
                        while po_queue:
                            po_queue.pop(0)()

    nc.compile()
    return nc


def _get_nc(causal: bool):
    if causal not in _compiled:
        _compiled[causal] = _build(causal)
    return _compiled[causal]


def kernel(x, freqs_cis, mask, wq, wk, wv, wo):
    import ml_dtypes
    from concourse.bass_utils import run_bass_kernel_spmd

    bf = ml_dtypes.bfloat16
    x = np.asarray(x, dtype=np.float32)
    freqs_cis = np.asarray(freqs_cis, dtype=np.float32)
    mask = np.asarray(mask, dtype=np.float32)
    wq = np.asarray(wq, dtype=np.float32)
    wk = np.asarray(wk, dtype=np.float32)
    wv = np.asarray(wv, dtype=np.float32)
    wo = np.asarray(wo, dtype=np.float32)

    tri = np.tril(np.ones((S, S), dtype=bool))
    causal = bool((mask[tri] == 0.0).all() and (mask[~tri] < -1e30).all())
    if not causal and not (mask == 0.0).all():
        return _numpy_ref(x, freqs_cis, mask, wq, wk, wv, wo)

    nc = _get_nc(causal)

    cos = freqs_cis[:, :, 0]
    sin = freqs_cis[:, :, 1]
    cosS = np.ascontiguousarray(cos.reshape(SB, 128, 64).transpose(1, 0, 2))
    sinS = np.ascontiguousarray(sin.reshape(SB, 128, 64).transpose(1, 0, 2))
    mtile = (np.ascontiguousarray(mask[0:128, 0:128].T) if causal
             else np.zeros((128, 128), dtype=np.float32))
    onest = np.ones((128, 128), dtype=bf)
    basist = np.ascontiguousarray(
        np.broadcast_to(np.eye(4, dtype=bf)[:, :, None], (4, 4, 128)))

    in_maps = []
    for c in range(8):
        b, i = c // 2, c % 2
        # x[b]: [S, D] -> [128 p, SB, KT, 128 j]
        xd = np.ascontiguousarray(
            x[b].reshape(SB, 128, KT, 128).transpose(3, 0, 2, 1).astype(bf))
        wcat = np.concatenate(
            [wq[1024 * i:1024 * (i + 1), :],
             wk[256 * i:256 * (i + 1), :],
             wv[256 * i:256 * (i + 1), :]], axis=0)  # [1536, D]
        wd = np.ascontiguousarray(
            wcat.T.reshape(KT, 128, NE * 128).transpose(1, 0, 2).astype(bf))
        wod = np.ascontiguousarray(
            wo[:, 1024 * i:1024 * (i + 1)].T.reshape(HPC, 128, D)
            .transpose(1, 0, 2).astype(bf))
        in_maps.append({
            "xd": xd, "wd": wd, "wod": wod,
            "cosS": cosS, "sinS": sinS, "mtile": mtile, "onest": onest,
            "basist": basist,
        })

    res = run_bass_kernel_spmd(nc, in_maps, core_ids=list(range(8)))
    out = np.empty((B, S, D), dtype=np.float32)
    for b in range(B):
        out[b] = res.results[2 * b]["outT"].T + res.results[2 * b + 1]["outT"].T
    return out


def _numpy_ref(x, freqs_cis, mask, wq, wk, wv, wo):
    xq = (x @ wq.T).reshape(B, S, H, HD)
    xk = (x @ wk.T).reshape(B, S, KV, HD)
    xv = (x @ wv.T).reshape(B, S, KV, HD)

    def rope(xh):
        x2 = xh.reshape(*xh.shape[:-1], HD // 2, 2)
        fc = freqs_cis[None, :, None, :, :]
        real = x2[..., 0] * fc[..., 0] - x2[..., 1] * fc[..., 1]
        imag = x2[..., 0] * fc[..., 1] + x2[..., 1] * fc[..., 0]
        return np.concatenate([real, imag], axis=-1)

    xq, xk = rope(xq), rope(xk)
    q = xq.reshape(B, S, KV, NREP, HD)
    sc = np.einsum('bqgrd,bkgd->bgrqk', q, xk) * SCALE + mask[None, None, None]
    sc = sc - sc.max(axis=-1, keepdims=True)
    p = np.exp(sc)
    p /= p.sum(axis=-1, keepdims=True)
    o = np.einsum('bgrqk,bkgd->bqgrd', p, xv).reshape(B, S, H * HD)
    return (o @ wo.T).astype(np.float32)


# revision 40
# speedup vs baseline: 1.2612x; 1.0076x over previous
"""Trainium2 Bass kernel for nn_Attention (B=4, S=2048, D=2048, H=16, KV=4, HD=128).

Sharding (8 cores): data-parallel over batch (4) x tensor-parallel over
KV-head-group halves (2). Core c handles batch b=c//2 and q-heads
[8*(c%2), 8*(c%2)+8) == kv groups {2*(c%2), 2*(c%2)+1}. Each core produces a
partial output (its heads' contribution through wo); the host sums the two
partials per batch.

All matmul operands are bf16 (PSUM accumulation stays f32): full PE speed at
any tile width, half the DMA bytes, and 1.0-rate PE transposes. Stage 1 is a
single fused pass over x: per s-block, one PSUM accumulation produces
q(8)+k(2)+v(2) head slots; RoPE is applied in [s, hd] layout, then PE
transposes write qT/kT ([hd, s]); v is copied raw. The transposes for s-block
i are emitted after the matmuls of s-block i+1 so the in-order PE never waits
on the DVE rope. Stage 2 computes attention transposed (scoresT[k,q]; kT
stationary, qT moving) so the ACT exp pass doubles as the PSUM->SBUF move.
Softmax denominators use tiny-output matmuls (probs block as stationary, ones
column moving -> [128q, 1] accumulated over k-blocks) instead of re-streaming
probs through a ones-row matmul; the per-head [128,4] reciprocal is
PE-transposed to [4,128] and broadcast to a [128,512] tile via basis-matrix
matmuls, and normalization is fused into the AV PSUM->SBUF move (one DVE
multiply). Each head's denominator tail is deferred by one head so the PE
never waits on it. wo stays resident in SBUF, and the out-projection of
q-superblock i is interleaved between the attention heads of q-superblock
i+1 (two 128-row output blocks per head), which keeps ACT busy with exps
during what used to be a PE-only out-projection phase.

Hardware notes learned the hard way: PSUM accumulation groups must be
exclusive per bank (interleaved open groups clobber each other on reset), so
the 4-column denominator tile is zero-initialized once per head and every
tiny matmul accumulates with start=False; DVE instructions may read only one
operand from PSUM (the reciprocal-broadcast tile is staged through SBUF); the
Pool engine cannot access PSUM at all. Startup streams sb0+sb1 fused so the
PE rides the 6MB weight DMA, and the last three s-blocks' transposes are
deferred into stage 2's ACT-bound first q-superblock.
"""
import numpy as np

B, S, D = 4, 2048, 2048
H, KV, HD = 16, 4, 128
NREP = H // KV
SCALE = float(HD) ** -0.5

SB = S // 128          # 16 s-blocks
KT = D // 128          # 16 contraction tiles for projections
QSB = S // 512         # 4 q-superblocks
HPC = 8                # q heads per core
GPC = 2                # kv groups per core
NE = HPC + 2 * GPC     # 12 projection head-slots per core (q0..7, k0, k1, v0, v1)
NR = HPC + GPC         # 10 slots that get RoPE

_compiled = {}


def _build(causal: bool):
    import concourse.bass as bass  # noqa: F401
    import concourse.tile as tile
    from concourse import bacc, mybir
    from concourse.masks import make_identity

    f32 = mybir.dt.float32
    bf16 = mybir.dt.bfloat16
    AF = mybir.ActivationFunctionType
    ALU = mybir.AluOpType

    nc = bacc.Bacc("TRN2")

    # x: [128, SB, KT, 128] with x_dram[p, sb, kt, j] = x[b, sb*128+j, kt*128+p]
    xd = nc.dram_tensor("xd", [128, SB, KT, 128], bf16, kind="ExternalInput")
    # fused qkv weights: wt[p, kt, e] = wcat[e, kt*128+p], e over 12*128
    wd = nc.dram_tensor("wd", [128, KT, NE * 128], bf16, kind="ExternalInput")
    # wo: wod[p, h, d] = wo[d, off + h*128 + p]
    wod = nc.dram_tensor("wod", [128, HPC, D], bf16, kind="ExternalInput")
    cosS = nc.dram_tensor("cosS", [128, SB, 64], f32, kind="ExternalInput")
    sinS = nc.dram_tensor("sinS", [128, SB, 64], f32, kind="ExternalInput")
    mtile = nc.dram_tensor("mtile", [128, 128], f32, kind="ExternalInput")
    onest = nc.dram_tensor("onest", [128, 128], bf16, kind="ExternalInput")
    # basis[k, qb, p] = 1.0 if k == qb else 0 (k, qb in 0..3)
    basist = nc.dram_tensor("basist", [4, 4, 128], bf16, kind="ExternalInput")
    outT = nc.dram_tensor("outT", [D, S], f32, kind="ExternalOutput")

    with tile.TileContext(nc) as tc:
        with tc.tile_pool(name="persist", bufs=1) as persist:
            qT = [persist.tile([128, S], bf16, tag=f"qT{h}", name=f"qT{h}") for h in range(HPC)]
            kT = [persist.tile([128, S], bf16, tag=f"kTg{g}", name=f"kTg{g}") for g in range(GPC)]
            vsb = [persist.tile([128, SB, 128], bf16, tag=f"v{g}", name=f"v{g}") for g in range(GPC)]
            msk = persist.tile([128, 128], f32, tag="msk")
            ones = persist.tile([128, 128], bf16, tag="ones")
            zer4 = persist.tile([128, 4], bf16, tag="zer4")
            basis = persist.tile([4, 4, 128], bf16, tag="basis")
            wo_sb = persist.tile([128, HPC, D], bf16, tag="wo")
            ident = persist.tile([128, 128], bf16, tag="ident")
            cos_t = persist.tile([128, SB, 64], f32, tag="cos")
            sin_t = persist.tile([128, SB, 64], f32, tag="sin")
            psb = persist.tile([128, NR, 128], f32, tag="psb")
            psb0 = persist.tile([128, NR, 128], f32, tag="psb0")
            psb1 = persist.tile([128, NR, 128], f32, tag="psb1")
            # late s-blocks' rope output lives in persistent tiles; their
            # transposes are emitted inside stage 2 (fills ACT-bound gaps)
            NLATE = 3
            rpLate = [persist.tile([128, NR, 128], bf16, tag=f"rpL{i}",
                                   name=f"rpL{i}") for i in range(NLATE)]

            # ------------ Stage 1: fused projections + RoPE + transposes ----
            s1ctx = tc.tile_pool(name="s1const", bufs=1)
            s1const = s1ctx.__enter__()
            ident_f = s1const.tile([128, 128], f32, tag="identf")
            make_identity(nc, ident_f)
            nc.vector.tensor_copy(out=ident, in_=ident_f)
            nc.vector.tensor_tensor(
                out=zer4, in0=ident[:, 0:4], in1=ident[:, 0:4],
                op=ALU.subtract)
            # preload the ACT exp table so the first real exp (stage 2)
            # doesn't pay the table-load latency
            warm = s1const.tile([1, 8], f32, tag="warm")
            nc.scalar.activation(
                out=warm, in_=ident_f[0:1, 0:8], func=AF.Exp, scale=1.0)

            EW = NE * 128  # 1536
            with tc.tile_pool(name="w1", bufs=1) as wpool, \
                 tc.tile_pool(name="xs1", bufs=3) as xpool, \
                 tc.tile_pool(name="rs1", bufs=3) as rpool, \
                 tc.tile_pool(name="pq1", bufs=2, space="PSUM") as pqp, \
                 tc.tile_pool(name="pt1", bufs=2, space="PSUM") as ptp:
                wt = wpool.tile([128, KT, EW], bf16, tag="wt")

                def make_trans(rp, sb_):
                    def emit():
                        for h in range(NR):
                            pt = ptp.tile([128, 128], bf16, tag="pt")
                            nc.tensor.transpose(pt, rp[:, h, :], ident)
                            dst = qT[h] if h < HPC else kT[h - HPC]
                            nc.vector.tensor_copy(
                                out=dst[:, sb_ * 128:(sb_ + 1) * 128], in_=pt)
                    return emit

                def emit_rope(ps, sb):
                    for g in range(GPC):
                        nc.scalar.copy(
                            out=vsb[g][:, sb, :], in_=ps[:, HPC + GPC + g, :])
                    if sb <= 1 or sb == SB - 1:
                        # copy q/k slots out of PSUM so the PSUM buffer frees
                        # early (sb0/sb1: the fused prefix holds both ps
                        # buffers; sb15: stage 2 reuses the banks)
                        dst_ps = (psb0, psb1, psb)[min(sb, 2)]
                        nc.scalar.copy(out=dst_ps, in_=ps[:, 0:NR, :])
                        src = dst_ps
                    else:
                        src = ps
                    if sb >= SB - NLATE:
                        rp = rpLate[sb - (SB - NLATE)]
                    else:
                        rp = rpool.tile([128, NR, 128], bf16, tag="rope",
                                        name="rope")
                    ev = src[:, 0:NR, 0:128:2]
                    od = src[:, 0:NR, 1:128:2]
                    cb = cos_t[:, None, sb, :].broadcast_to([128, NR, 64])
                    sn = sin_t[:, None, sb, :].broadcast_to([128, NR, 64])
                    t1 = rpool.tile([128, NR, 64], f32, tag="t1", name="t1")
                    t2 = rpool.tile([128, NR, 64], f32, tag="t2", name="t2")
                    nc.vector.tensor_tensor(out=t1, in0=ev, in1=cb, op=ALU.mult)
                    nc.vector.tensor_tensor(out=t2, in0=od, in1=sn, op=ALU.mult)
                    nc.vector.tensor_tensor(
                        out=rp[:, :, 0:64], in0=t1, in1=t2, op=ALU.subtract)
                    nc.vector.tensor_tensor(out=t1, in0=ev, in1=sn, op=ALU.mult)
                    nc.vector.tensor_tensor(out=t2, in0=od, in1=cb, op=ALU.mult)
                    nc.vector.tensor_tensor(
                        out=rp[:, :, 64:128], in0=t1, in1=t2, op=ALU.add)
                    return rp

                def emit_mm(ps2, xs, kt):
                    for n0 in range(0, EW, 512):
                        nc.tensor.matmul(
                            ps2[:, n0:n0 + 512], xs[:, kt, :],
                            wt[:, kt, n0:n0 + 512],
                            start=(kt == 0), stop=(kt == KT - 1))

                pending = []
                # fused prefix: sb0+sb1 interleaved per weight chunk so the
                # PE has two s-blocks of work while the wt DMA streams in
                xs0 = xpool.tile([128, KT, 128], bf16, tag="xs", name="xs")
                xs1 = xpool.tile([128, KT, 128], bf16, tag="xs", name="xs")
                nc.sync.dma_start(out=xs0[:, 0:4, :], in_=xd[:, 0, 0:4, :])
                nc.sync.dma_start(out=xs1[:, 0:4, :], in_=xd[:, 1, 0:4, :])
                nc.sync.dma_start(out=wt[:, 0, :], in_=wd[:, 0, :])
                nc.sync.dma_start(out=wt[:, 1, :], in_=wd[:, 1, :])
                nc.sync.dma_start(out=wt[:, 2, :], in_=wd[:, 2, :])
                nc.sync.dma_start(out=xs0[:, 4:KT, :], in_=xd[:, 0, 4:KT, :])
                nc.sync.dma_start(out=xs1[:, 4:KT, :], in_=xd[:, 1, 4:KT, :])
                for kt in range(3, 6):
                    nc.sync.dma_start(out=wt[:, kt, :], in_=wd[:, kt, :])
                nc.sync.dma_start(out=cos_t, in_=cosS[:, :, :])
                nc.sync.dma_start(out=sin_t, in_=sinS[:, :, :])
                for kt in range(6, KT):
                    nc.sync.dma_start(out=wt[:, kt, :], in_=wd[:, kt, :])
                nc.sync.dma_start(out=msk, in_=mtile[:, :])
                nc.sync.dma_start(out=ones, in_=onest[:, :])
                nc.sync.dma_start(out=basis, in_=basist[:, :, :])
                ps0 = pqp.tile([128, NE, 128], f32, tag="ps", name="ps")
                ps1 = pqp.tile([128, NE, 128], f32, tag="ps", name="ps")
                ps0f = ps0.rearrange("p h d -> p (h d)")
                ps1f = ps1.rearrange("p h d -> p (h d)")
                for kt in range(KT):
                    emit_mm(ps0f, xs0, kt)
                    emit_mm(ps1f, xs1, kt)
                rp0 = emit_rope(ps0, 0)
                rp1 = emit_rope(ps1, 1)
                pending.append(make_trans(rp0, 0))
                pending.append(make_trans(rp1, 1))

                for sb in range(2, SB):
                    xs = xpool.tile([128, KT, 128], bf16, tag="xs", name="xs")
                    nc.sync.dma_start(out=xs, in_=xd[:, sb, :, :])
                    if sb == SB - 1:
                        # wo arrives during the tail of stage 1
                        nc.sync.dma_start(out=wo_sb, in_=wod[:, :, :])
                    ps = pqp.tile([128, NE, 128], f32, tag="ps", name="ps")
                    ps2 = ps.rearrange("p h d -> p (h d)")
                    for kt in range(KT):
                        emit_mm(ps2, xs, kt)
                    # transposes of an earlier s-block (rope long done)
                    if pending:
                        pending.pop(0)()
                    rp = emit_rope(ps, sb)
                    if sb < SB - NLATE:
                        pending.append(make_trans(rp, sb))
                while pending:
                    pending.pop(0)()
            s1ctx.__exit__(None, None, None)

            # ------------ Stage 2: attention (scoresT) + out-projection -----
            with tc.tile_pool(name="pr2", bufs=3) as prpool, \
                 tc.tile_pool(name="att2", bufs=3) as attpool, \
                 tc.tile_pool(name="dn2", bufs=3) as dnpool, \
                 tc.tile_pool(name="o2", bufs=3) as opool, \
                 tc.tile_pool(name="psc", bufs=3, space="PSUM") as pscp, \
                 tc.tile_pool(name="pav", bufs=2, space="PSUM") as pavp, \
                 tc.tile_pool(name="pds", bufs=1, space="PSUM") as pdsp, \
                 tc.tile_pool(name="scr", bufs=2, space="PSUM") as scrp:

                def make_transL(i_, j_, on_act_):
                    def emit():
                        pt = scrp.tile([128, 128], bf16, tag="scr", name="ptL")
                        nc.tensor.transpose(pt, rpLate[i_][:, j_, :], ident)
                        dst = qT[j_] if j_ < HPC else kT[j_ - HPC]
                        sbL = SB - NLATE + i_
                        # alternate the PSUM->SBUF copies between ACT and DVE
                        # to balance the two engines in the ACT-bound qsb0
                        if on_act_:
                            nc.scalar.copy(
                                out=dst[:, sbL * 128:(sbL + 1) * 128], in_=pt)
                        else:
                            nc.vector.tensor_copy(
                                out=dst[:, sbL * 128:(sbL + 1) * 128], in_=pt)
                    return emit

                transL = [make_transL(i, j, (i * NR + j) % 2 == 0)
                          for i in range(NLATE) for j in range(NR)]

                def make_tail(h_, av_, rrh_, att_, qsb_):
                    # part a: transpose the reciprocal row; copy it to SBUF on
                    # DVE (its backlog is far shorter than ACT's exp queue)
                    rrow = [None]

                    def emit_a():
                        trp = scrp.tile([4, 128], bf16, tag="scr")
                        nc.tensor.transpose(trp, rrh_, ident)
                        rrow[0] = dnpool.tile(
                            [4, 128], bf16, tag="rrow", name="rrow")
                        nc.vector.tensor_copy(out=rrow[0], in_=trp)

                    def emit_b():
                        rps = scrp.tile([128, 512], f32, tag="scr")
                        for qb in range(4):
                            nc.tensor.matmul(
                                rps[:, qb * 128:(qb + 1) * 128],
                                basis[:, qb, :], rrow[0],
                                start=True, stop=True)
                        # DVE may read only ONE operand from PSUM: stage the
                        # broadcast tile to SBUF (bf16), then multiply it into
                        # the AV PSUM on the way to the att tile. In qsb0 the
                        # DVE is the regional bottleneck, so stage on ACT there
                        rsb = dnpool.tile([128, 512], bf16, tag="rsb")
                        if qsb_ == 0:
                            nc.scalar.copy(out=rsb, in_=rps)
                        else:
                            nc.vector.tensor_copy(out=rsb, in_=rps)
                        nc.vector.tensor_tensor(
                            out=att_[:, h_, :], in0=av_, in1=rsb,
                            op=ALU.mult)
                    return emit_a, emit_b

                def make_po(att_, qsb_, m_):
                    def emit():
                        po = scrp.tile([128, 512], f32, tag="scr")
                        for e in range(HPC):
                            nc.tensor.matmul(
                                po, wo_sb[:, e, m_ * 128:(m_ + 1) * 128],
                                att_[:, e, :],
                                start=(e == 0), stop=(e == HPC - 1))
                        ot = opool.tile([128, 512], f32, tag="ot")
                        nc.vector.tensor_copy(out=ot, in_=po)
                        nc.sync.dma_start(
                            out=outT[m_ * 128:(m_ + 1) * 128,
                                     qsb_ * 512:(qsb_ + 1) * 512],
                            in_=ot)
                    return emit

                po_queue = []
                tail_prev = None
                for qsb in range(QSB):
                    att = attpool.tile([128, HPC, 512], bf16, tag="att")
                    maxkt = (qsb + 1) * 4 if causal else SB
                    q0g = qsb * 512
                    for g in range(GPC):
                        for r in range(NREP):
                            h = g * NREP + r
                            probs = prpool.tile([128, SB, 512], bf16, tag="probs")
                            dsT = pdsp.tile([128, 4], f32, tag="dsT")
                            av = pavp.tile([128, 512], f32, tag="av")
                            # PSUM accumulation groups must be exclusive per
                            # bank on HW: zero-init the 4-column denominator
                            # tile and accumulate with start=False throughout
                            nc.vector.tensor_copy(out=dsT, in_=zer4)
                            for t in range(maxkt):
                                ql = max(0, t * 128 - q0g) if causal else 0
                                sc = pscp.tile([128, 512], f32, tag="sc")
                                nc.tensor.matmul(
                                    sc[:, ql:512],
                                    kT[g][:, t * 128:(t + 1) * 128],
                                    qT[h][:, q0g + ql:q0g + 512],
                                    start=True, stop=True)
                                is_diag = causal and t * 128 >= q0g
                                if is_diag:
                                    # add mask pre-scale: exp(SCALE*(sc+msk))
                                    # == exp(SCALE*sc + mask) for the 0/-inf
                                    # mask (underflows to 0 identically)
                                    nc.vector.tensor_tensor(
                                        out=sc[:, ql:ql + 128],
                                        in0=sc[:, ql:ql + 128],
                                        in1=msk, op=ALU.add)
                                nc.scalar.activation(
                                    out=probs[:, t, ql:512],
                                    in_=sc[:, ql:512], func=AF.Exp,
                                    scale=SCALE)
                                nc.tensor.matmul(
                                    av[:, ql:512], vsb[g][:, t, :],
                                    probs[:, t, ql:512],
                                    start=(t == 0), stop=(t == maxkt - 1),
                                    skip_group_check=True)
                                # per-q-block denominator partials: tiny-output
                                # matmuls (probs block stationary, ones moving)
                                qb0 = max(0, t - qsb * 4) if causal else 0
                                for qb in range(qb0, 4):
                                    tlast = qsb * 4 + qb if causal else maxkt - 1
                                    nc.tensor.matmul(
                                        dsT[:, qb:qb + 1],
                                        probs[:, t, qb * 128:(qb + 1) * 128],
                                        ones[:, 0:1],
                                        start=False, stop=(t == tlast),
                                        skip_group_check=True)
                            rrh = dnpool.tile([128, 4], bf16, tag="rrh")
                            with nc.allow_low_precision(reason="softmax recip"):
                                nc.vector.reciprocal(out=rrh, in_=dsT)
                            # deferred work: previous head's denominator tail
                            # interleaved with two out-proj blocks of the
                            # previous qsb (the po blocks cover the Pool-copy
                            # latency and let ACT drain its exp backlog)
                            if tail_prev is not None:
                                tail_prev[0]()
                            if po_queue:
                                po_queue.pop(0)()
                            if tail_prev is not None:
                                tail_prev[1]()
                            if po_queue:
                                po_queue.pop(0)()
                            if qsb == 0:
                                for _ in range(3):
                                    if transL:
                                        transL.pop(0)()
                            elif qsb == 1:
                                if transL:
                                    transL.pop(0)()
                            tail_prev = make_tail(h, av, rrh, att, qsb)
                    tail_prev[0]()
                    tail_prev[1]()
                    tail_prev = None
                    po_queue = [make_po(att, qsb, m) for m in range(KT)]
                    if qsb == QSB - 1:
                        while po_queue:
                            po_queue.pop(0)()

    nc.compile()
    return nc


def _get_nc(causal: bool):
    if causal not in _compiled:
        _compiled[causal] = _build(causal)
    return _compiled[causal]


def kernel(x, freqs_cis, mask, wq, wk, wv, wo):
    import ml_dtypes
    from concourse.bass_utils import run_bass_kernel_spmd

    bf = ml_dtypes.bfloat16
    x = np.asarray(x, dtype=np.float32)
    freqs_cis = np.asarray(freqs_cis, dtype=np.float32)
    mask = np.asarray(mask, dtype=np.float32)
    wq = np.asarray(wq, dtype=np.float32)
    wk = np.asarray(wk, dtype=np.float32)
    wv = np.asarray(wv, dtype=np.float32)
    wo = np.asarray(wo, dtype=np.float32)

    tri = np.tril(np.ones((S, S), dtype=bool))
    causal = bool((mask[tri] == 0.0).all() and (mask[~tri] < -1e30).all())
    if not causal and not (mask == 0.0).all():
        return _numpy_ref(x, freqs_cis, mask, wq, wk, wv, wo)

    nc = _get_nc(causal)

    cos = freqs_cis[:, :, 0]
    sin = freqs_cis[:, :, 1]
    cosS = np.ascontiguousarray(cos.reshape(SB, 128, 64).transpose(1, 0, 2))
    sinS = np.ascontiguousarray(sin.reshape(SB, 128, 64).transpose(1, 0, 2))
    mtile = (np.ascontiguousarray(mask[0:128, 0:128].T) if causal
             else np.zeros((128, 128), dtype=np.float32))
    onest = np.ones((128, 128), dtype=bf)
    basist = np.ascontiguousarray(
        np.broadcast_to(np.eye(4, dtype=bf)[:, :, None], (4, 4, 128)))

    in_maps = []
    for c in range(8):
        b, i = c // 2, c % 2
        # x[b]: [S, D] -> [128 p, SB, KT, 128 j]
        xd = np.ascontiguousarray(
            x[b].reshape(SB, 128, KT, 128).transpose(3, 0, 2, 1).astype(bf))
        wcat = np.concatenate(
            [wq[1024 * i:1024 * (i + 1), :],
             wk[256 * i:256 * (i + 1), :],
             wv[256 * i:256 * (i + 1), :]], axis=0)  # [1536, D]
        wd = np.ascontiguousarray(
            wcat.T.reshape(KT, 128, NE * 128).transpose(1, 0, 2).astype(bf))
        wod = np.ascontiguousarray(
            wo[:, 1024 * i:1024 * (i + 1)].T.reshape(HPC, 128, D)
            .transpose(1, 0, 2).astype(bf))
        in_maps.append({
            "xd": xd, "wd": wd, "wod": wod,
            "cosS": cosS, "sinS": sinS, "mtile": mtile, "onest": onest,
            "basist": basist,
        })

    res = run_bass_kernel_spmd(nc, in_maps, core_ids=list(range(8)))
    out = np.empty((B, S, D), dtype=np.float32)
    for b in range(B):
        out[b] = res.results[2 * b]["outT"].T + res.results[2 * b + 1]["outT"].T
    return out


def _numpy_ref(x, freqs_cis, mask, wq, wk, wv, wo):
    xq = (x @ wq.T).reshape(B, S, H, HD)
    xk = (x @ wk.T).reshape(B, S, KV, HD)
    xv = (x @ wv.T).reshape(B, S, KV, HD)

    def rope(xh):
        x2 = xh.reshape(*xh.shape[:-1], HD // 2, 2)
        fc = freqs_cis[None, :, None, :, :]
        real = x2[..., 0] * fc[..., 0] - x2[..., 1] * fc[..., 1]
        imag = x2[..., 0] * fc[..., 1] + x2[..., 1] * fc[..., 0]
        return np.concatenate([real, imag], axis=-1)

    xq, xk = rope(xq), rope(xk)
    q = xq.reshape(B, S, KV, NREP, HD)
    sc = np.einsum('bqgrd,bkgd->bgrqk', q, xk) * SCALE + mask[None, None, None]
    sc = sc - sc.max(axis=-1, keepdims=True)
    p = np.exp(sc)
    p /= p.sum(axis=-1, keepdims=True)
    o = np.einsum('bgrqk,bkgd->bqgrd', p, xv).reshape(B, S, H * HD)
    return (o @ wo.T).astype(np.float32)


# revision 45
# speedup vs baseline: 1.2626x; 1.0011x over previous
"""Trainium2 Bass kernel for nn_Attention (B=4, S=2048, D=2048, H=16, KV=4, HD=128).

Sharding (8 cores): data-parallel over batch (4) x tensor-parallel over
KV-head-group halves (2). Core c handles batch b=c//2 and q-heads
[8*(c%2), 8*(c%2)+8) == kv groups {2*(c%2), 2*(c%2)+1}. Each core produces a
partial output (its heads' contribution through wo); the host sums the two
partials per batch.

All matmul operands are bf16 (PSUM accumulation stays f32): full PE speed at
any tile width, half the DMA bytes, and 1.0-rate PE transposes. Stage 1 is a
single fused pass over x: per s-block, one PSUM accumulation produces
q(8)+k(2)+v(2) head slots; RoPE is applied in [s, hd] layout, then PE
transposes write qT/kT ([hd, s]); v is copied raw. The transposes for s-block
i are emitted after the matmuls of s-block i+1 so the in-order PE never waits
on the DVE rope. Stage 2 computes attention transposed (scoresT[k,q]; kT
stationary, qT moving) so the ACT exp pass doubles as the PSUM->SBUF move.
Softmax denominators use tiny-output matmuls (probs block as stationary, ones
column moving -> [128q, 1] accumulated over k-blocks) instead of re-streaming
probs through a ones-row matmul; the per-head [128,4] reciprocal is
PE-transposed to [4,128] and broadcast to a [128,512] tile via basis-matrix
matmuls, and normalization is fused into the AV PSUM->SBUF move (one DVE
multiply). Each head's denominator tail is deferred by one head so the PE
never waits on it. wo stays resident in SBUF, and the out-projection of
q-superblock i is interleaved between the attention heads of q-superblock
i+1 (two 128-row output blocks per head), which keeps ACT busy with exps
during what used to be a PE-only out-projection phase.

Hardware notes learned the hard way: PSUM accumulation groups must be
exclusive per bank (interleaved open groups clobber each other on reset), so
the 4-column denominator tile is zero-initialized once per head and every
tiny matmul accumulates with start=False; DVE instructions may read only one
operand from PSUM (the reciprocal-broadcast tile is staged through SBUF); the
Pool engine cannot access PSUM at all. Startup streams sb0+sb1 fused so the
PE rides the 6MB weight DMA, and the last three s-blocks' transposes are
deferred into stage 2's ACT-bound first q-superblock.
"""
import numpy as np

B, S, D = 4, 2048, 2048
H, KV, HD = 16, 4, 128
NREP = H // KV
SCALE = float(HD) ** -0.5

SB = S // 128          # 16 s-blocks
KT = D // 128          # 16 contraction tiles for projections
QSB = S // 512         # 4 q-superblocks
HPC = 8                # q heads per core
GPC = 2                # kv groups per core
NE = HPC + 2 * GPC     # 12 projection head-slots per core (q0..7, k0, k1, v0, v1)
NR = HPC + GPC         # 10 slots that get RoPE

_compiled = {}


def _build(causal: bool):
    import concourse.bass as bass  # noqa: F401
    import concourse.tile as tile
    from concourse import bacc, mybir
    from concourse.masks import make_identity

    f32 = mybir.dt.float32
    bf16 = mybir.dt.bfloat16
    AF = mybir.ActivationFunctionType
    ALU = mybir.AluOpType

    nc = bacc.Bacc("TRN2")

    # x: [128, SB, KT, 128] with x_dram[p, sb, kt, j] = x[b, sb*128+j, kt*128+p]
    xd = nc.dram_tensor("xd", [128, SB, KT, 128], bf16, kind="ExternalInput")
    # fused qkv weights: wt[p, kt, e] = wcat[e, kt*128+p], e over 12*128
    wd = nc.dram_tensor("wd", [128, KT, NE * 128], bf16, kind="ExternalInput")
    # wo: wod[p, h, d] = wo[d, off + h*128 + p]
    wod = nc.dram_tensor("wod", [128, HPC, D], bf16, kind="ExternalInput")
    cosS = nc.dram_tensor("cosS", [128, SB, 64], f32, kind="ExternalInput")
    sinS = nc.dram_tensor("sinS", [128, SB, 64], f32, kind="ExternalInput")
    mtile = nc.dram_tensor("mtile", [128, 128], f32, kind="ExternalInput")
    onest = nc.dram_tensor("onest", [128, 128], bf16, kind="ExternalInput")
    # basis[k, qb, p] = 1.0 if k == qb else 0 (k, qb in 0..3)
    basist = nc.dram_tensor("basist", [4, 4, 128], bf16, kind="ExternalInput")
    outT = nc.dram_tensor("outT", [D, S], f32, kind="ExternalOutput")

    with tile.TileContext(nc) as tc:
        with tc.tile_pool(name="persist", bufs=1) as persist:
            qT = [persist.tile([128, S], bf16, tag=f"qT{h}", name=f"qT{h}") for h in range(HPC)]
            kT = [persist.tile([128, S], bf16, tag=f"kTg{g}", name=f"kTg{g}") for g in range(GPC)]
            vsb = [persist.tile([128, SB, 128], bf16, tag=f"v{g}", name=f"v{g}") for g in range(GPC)]
            msk = persist.tile([128, 128], f32, tag="msk")
            ones = persist.tile([128, 128], bf16, tag="ones")
            zer4 = persist.tile([128, 4], bf16, tag="zer4")
            basis = persist.tile([4, 4, 128], bf16, tag="basis")
            wo_sb = persist.tile([128, HPC, D], bf16, tag="wo")
            ident = persist.tile([128, 128], bf16, tag="ident")
            cos_t = persist.tile([128, SB, 64], f32, tag="cos")
            sin_t = persist.tile([128, SB, 64], f32, tag="sin")
            psb = persist.tile([128, NR, 128], f32, tag="psb")
            psb0 = persist.tile([128, NR, 128], f32, tag="psb0")
            psb1 = persist.tile([128, NR, 128], f32, tag="psb1")
            # late s-blocks' rope output lives in persistent tiles; their
            # transposes are emitted inside stage 2 (fills ACT-bound gaps)
            NLATE = 3
            rpLate = [persist.tile([128, NR, 128], bf16, tag=f"rpL{i}",
                                   name=f"rpL{i}") for i in range(NLATE)]

            # ------------ Stage 1: fused projections + RoPE + transposes ----
            s1ctx = tc.tile_pool(name="s1const", bufs=1)
            s1const = s1ctx.__enter__()
            ident_f = s1const.tile([128, 128], f32, tag="identf")
            make_identity(nc, ident_f)
            nc.vector.tensor_copy(out=ident, in_=ident_f)
            nc.vector.tensor_tensor(
                out=zer4, in0=ident[:, 0:4], in1=ident[:, 0:4],
                op=ALU.subtract)
            # preload the ACT exp table so the first real exp (stage 2)
            # doesn't pay the table-load latency
            warm = s1const.tile([1, 8], f32, tag="warm")
            nc.scalar.activation(
                out=warm, in_=ident_f[0:1, 0:8], func=AF.Exp, scale=1.0)

            EW = NE * 128  # 1536
            with tc.tile_pool(name="w1", bufs=1) as wpool, \
                 tc.tile_pool(name="xs1", bufs=3) as xpool, \
                 tc.tile_pool(name="rs1", bufs=3) as rpool, \
                 tc.tile_pool(name="pq1", bufs=2, space="PSUM") as pqp, \
                 tc.tile_pool(name="pt1", bufs=2, space="PSUM") as ptp:
                wt = wpool.tile([128, KT, EW], bf16, tag="wt")

                def make_trans(rp, sb_):
                    def emit():
                        for h in range(NR):
                            pt = ptp.tile([128, 128], bf16, tag="pt")
                            nc.tensor.transpose(pt, rp[:, h, :], ident)
                            dst = qT[h] if h < HPC else kT[h - HPC]
                            nc.vector.tensor_copy(
                                out=dst[:, sb_ * 128:(sb_ + 1) * 128], in_=pt)
                    return emit

                def emit_rope(ps, sb):
                    for g in range(GPC):
                        nc.scalar.copy(
                            out=vsb[g][:, sb, :], in_=ps[:, HPC + GPC + g, :])
                    if sb <= 1 or sb == SB - 1:
                        # copy q/k slots out of PSUM so the PSUM buffer frees
                        # early (sb0/sb1: the fused prefix holds both ps
                        # buffers; sb15: stage 2 reuses the banks)
                        dst_ps = (psb0, psb1, psb)[min(sb, 2)]
                        nc.scalar.copy(out=dst_ps, in_=ps[:, 0:NR, :])
                        src = dst_ps
                    else:
                        src = ps
                    if sb >= SB - NLATE:
                        rp = rpLate[sb - (SB - NLATE)]
                    else:
                        rp = rpool.tile([128, NR, 128], bf16, tag="rope",
                                        name="rope")
                    ev = src[:, 0:NR, 0:128:2]
                    od = src[:, 0:NR, 1:128:2]
                    cb = cos_t[:, None, sb, :].broadcast_to([128, NR, 64])
                    sn = sin_t[:, None, sb, :].broadcast_to([128, NR, 64])
                    t1 = rpool.tile([128, NR, 64], f32, tag="t1", name="t1")
                    t2 = rpool.tile([128, NR, 64], f32, tag="t2", name="t2")
                    nc.vector.tensor_tensor(out=t1, in0=ev, in1=cb, op=ALU.mult)
                    nc.vector.tensor_tensor(out=t2, in0=od, in1=sn, op=ALU.mult)
                    nc.vector.tensor_tensor(
                        out=rp[:, :, 0:64], in0=t1, in1=t2, op=ALU.subtract)
                    nc.vector.tensor_tensor(out=t1, in0=ev, in1=sn, op=ALU.mult)
                    nc.vector.tensor_tensor(out=t2, in0=od, in1=cb, op=ALU.mult)
                    nc.vector.tensor_tensor(
                        out=rp[:, :, 64:128], in0=t1, in1=t2, op=ALU.add)
                    return rp

                def emit_mm(ps2, xs, kt):
                    for n0 in range(0, EW, 512):
                        nc.tensor.matmul(
                            ps2[:, n0:n0 + 512], xs[:, kt, :],
                            wt[:, kt, n0:n0 + 512],
                            start=(kt == 0), stop=(kt == KT - 1))

                pending = []
                # fused prefix: sb0+sb1 interleaved per weight chunk so the
                # PE has two s-blocks of work while the wt DMA streams in
                xs0 = xpool.tile([128, KT, 128], bf16, tag="xs", name="xs")
                xs1 = xpool.tile([128, KT, 128], bf16, tag="xs", name="xs")
                nc.sync.dma_start(out=xs0[:, 0:4, :], in_=xd[:, 0, 0:4, :])
                nc.sync.dma_start(out=xs1[:, 0:4, :], in_=xd[:, 1, 0:4, :])
                nc.sync.dma_start(out=wt[:, 0, :], in_=wd[:, 0, :])
                nc.sync.dma_start(out=wt[:, 1, :], in_=wd[:, 1, :])
                nc.sync.dma_start(out=wt[:, 2, :], in_=wd[:, 2, :])
                nc.sync.dma_start(out=xs0[:, 4:KT, :], in_=xd[:, 0, 4:KT, :])
                nc.sync.dma_start(out=xs1[:, 4:KT, :], in_=xd[:, 1, 4:KT, :])
                for kt in range(3, 6):
                    nc.sync.dma_start(out=wt[:, kt, :], in_=wd[:, kt, :])
                nc.sync.dma_start(out=cos_t, in_=cosS[:, :, :])
                nc.sync.dma_start(out=sin_t, in_=sinS[:, :, :])
                for kt in range(6, KT):
                    nc.sync.dma_start(out=wt[:, kt, :], in_=wd[:, kt, :])
                nc.sync.dma_start(out=msk, in_=mtile[:, :])
                nc.sync.dma_start(out=ones, in_=onest[:, :])
                nc.sync.dma_start(out=basis, in_=basist[:, :, :])
                ps0 = pqp.tile([128, NE, 128], f32, tag="ps", name="ps")
                ps1 = pqp.tile([128, NE, 128], f32, tag="ps", name="ps")
                ps0f = ps0.rearrange("p h d -> p (h d)")
                ps1f = ps1.rearrange("p h d -> p (h d)")
                for kt in range(KT):
                    emit_mm(ps0f, xs0, kt)
                    emit_mm(ps1f, xs1, kt)
                rp0 = emit_rope(ps0, 0)
                rp1 = emit_rope(ps1, 1)
                pending.append(make_trans(rp0, 0))
                pending.append(make_trans(rp1, 1))

                for sb in range(2, SB):
                    xs = xpool.tile([128, KT, 128], bf16, tag="xs", name="xs")
                    nc.sync.dma_start(out=xs, in_=xd[:, sb, :, :])
                    if sb == SB - 1:
                        # wo arrives during the tail of stage 1
                        nc.sync.dma_start(out=wo_sb, in_=wod[:, :, :])
                    ps = pqp.tile([128, NE, 128], f32, tag="ps", name="ps")
                    ps2 = ps.rearrange("p h d -> p (h d)")
                    for kt in range(KT):
                        emit_mm(ps2, xs, kt)
                    # transposes of an earlier s-block (rope long done)
                    if pending:
                        pending.pop(0)()
                    rp = emit_rope(ps, sb)
                    if sb < SB - NLATE:
                        pending.append(make_trans(rp, sb))
                while pending:
                    pending.pop(0)()
            s1ctx.__exit__(None, None, None)

            # ------------ Stage 2: attention (scoresT) + out-projection -----
            with tc.tile_pool(name="pr2", bufs=3) as prpool, \
                 tc.tile_pool(name="att2", bufs=3) as attpool, \
                 tc.tile_pool(name="dn2", bufs=3) as dnpool, \
                 tc.tile_pool(name="o2", bufs=3) as opool, \
                 tc.tile_pool(name="psc", bufs=3, space="PSUM") as pscp, \
                 tc.tile_pool(name="pav", bufs=2, space="PSUM") as pavp, \
                 tc.tile_pool(name="pds", bufs=1, space="PSUM") as pdsp, \
                 tc.tile_pool(name="scr", bufs=2, space="PSUM") as scrp:

                def make_transL(i_, j_, on_act_):
                    def emit():
                        pt = scrp.tile([128, 128], bf16, tag="scr", name="ptL")
                        nc.tensor.transpose(pt, rpLate[i_][:, j_, :], ident)
                        dst = qT[j_] if j_ < HPC else kT[j_ - HPC]
                        sbL = SB - NLATE + i_
                        # alternate the PSUM->SBUF copies between ACT and DVE
                        # to balance the two engines in the ACT-bound qsb0
                        if on_act_:
                            nc.scalar.copy(
                                out=dst[:, sbL * 128:(sbL + 1) * 128], in_=pt)
                        else:
                            nc.vector.tensor_copy(
                                out=dst[:, sbL * 128:(sbL + 1) * 128], in_=pt)
                    return emit

                transL = [make_transL(i, j, (i * NR + j) % 2 == 0)
                          for i in range(NLATE) for j in range(NR)]

                def make_tail(h_, av_, rrh_, att_, qsb_):
                    # part a: transpose the reciprocal row; copy it to SBUF on
                    # DVE (its backlog is far shorter than ACT's exp queue)
                    rrow = [None]

                    def emit_a():
                        trp = scrp.tile([4, 128], bf16, tag="scr")
                        nc.tensor.transpose(trp, rrh_, ident)
                        rrow[0] = dnpool.tile(
                            [4, 128], bf16, tag="rrow", name="rrow")
                        nc.vector.tensor_copy(out=rrow[0], in_=trp)

                    def emit_b():
                        rps = scrp.tile([128, 512], f32, tag="scr")
                        for qb in range(4):
                            nc.tensor.matmul(
                                rps[:, qb * 128:(qb + 1) * 128],
                                basis[:, qb, :], rrow[0],
                                start=True, stop=True)
                        # DVE may read only ONE operand from PSUM: stage the
                        # broadcast tile to SBUF (bf16), then multiply it into
                        # the AV PSUM on the way to the att tile. In qsb0 the
                        # DVE is the regional bottleneck, so stage on ACT there
                        rsb = dnpool.tile([128, 512], bf16, tag="rsb")
                        if qsb_ == 0:
                            nc.scalar.copy(out=rsb, in_=rps)
                        else:
                            nc.vector.tensor_copy(out=rsb, in_=rps)
                        nc.vector.tensor_tensor(
                            out=att_[:, h_, :], in0=av_, in1=rsb,
                            op=ALU.mult)
                    return emit_a, emit_b

                def make_po(att_, qsb_, m_, q0_=0, qw_=512):
                    def emit():
                        po = scrp.tile([128, 512], f32, tag="scr")
                        for e in range(HPC):
                            nc.tensor.matmul(
                                po[:, 0:qw_],
                                wo_sb[:, e, m_ * 128:(m_ + 1) * 128],
                                att_[:, e, q0_:q0_ + qw_],
                                start=(e == 0), stop=(e == HPC - 1))
                        ot = opool.tile([128, 512], f32, tag="ot")
                        nc.vector.tensor_copy(out=ot[:, 0:qw_], in_=po[:, 0:qw_])
                        nc.sync.dma_start(
                            out=outT[m_ * 128:(m_ + 1) * 128,
                                     qsb_ * 512 + q0_:qsb_ * 512 + q0_ + qw_],
                            in_=ot[:, 0:qw_])
                    return emit

                po_queue = []
                tail_prev = None
                for qsb in range(QSB):
                    att = attpool.tile([128, HPC, 512], bf16, tag="att")
                    maxkt = (qsb + 1) * 4 if causal else SB
                    q0g = qsb * 512
                    for g in range(GPC):
                        for r in range(NREP):
                            h = g * NREP + r
                            probs = prpool.tile([128, SB, 512], bf16, tag="probs")
                            dsT = pdsp.tile([128, 4], f32, tag="dsT")
                            av = pavp.tile([128, 512], f32, tag="av")
                            # PSUM accumulation groups must be exclusive per
                            # bank on HW: zero-init the 4-column denominator
                            # tile and accumulate with start=False throughout
                            nc.vector.tensor_copy(out=dsT, in_=zer4)
                            for t in range(maxkt):
                                ql = max(0, t * 128 - q0g) if causal else 0
                                sc = pscp.tile([128, 512], f32, tag="sc")
                                nc.tensor.matmul(
                                    sc[:, ql:512],
                                    kT[g][:, t * 128:(t + 1) * 128],
                                    qT[h][:, q0g + ql:q0g + 512],
                                    start=True, stop=True)
                                is_diag = causal and t * 128 >= q0g
                                if is_diag:
                                    # add mask pre-scale: exp(SCALE*(sc+msk))
                                    # == exp(SCALE*sc + mask) for the 0/-inf
                                    # mask (underflows to 0 identically)
                                    nc.vector.tensor_tensor(
                                        out=sc[:, ql:ql + 128],
                                        in0=sc[:, ql:ql + 128],
                                        in1=msk, op=ALU.add)
                                nc.scalar.activation(
                                    out=probs[:, t, ql:512],
                                    in_=sc[:, ql:512], func=AF.Exp,
                                    scale=SCALE)
                                nc.tensor.matmul(
                                    av[:, ql:512], vsb[g][:, t, :],
                                    probs[:, t, ql:512],
                                    start=(t == 0), stop=(t == maxkt - 1),
                                    skip_group_check=True)
                                # per-q-block denominator partials: tiny-output
                                # matmuls (probs block stationary, ones moving)
                                qb0 = max(0, t - qsb * 4) if causal else 0
                                for qb in range(qb0, 4):
                                    tlast = qsb * 4 + qb if causal else maxkt - 1
                                    nc.tensor.matmul(
                                        dsT[:, qb:qb + 1],
                                        probs[:, t, qb * 128:(qb + 1) * 128],
                                        ones[:, 0:1],
                                        start=False, stop=(t == tlast),
                                        skip_group_check=True)
                            rrh = dnpool.tile([128, 4], bf16, tag="rrh")
                            with nc.allow_low_precision(reason="softmax recip"):
                                nc.vector.reciprocal(out=rrh, in_=dsT)
                            # deferred work: previous head's denominator tail
                            # interleaved with two out-proj blocks of the
                            # previous qsb (the po blocks cover the Pool-copy
                            # latency and let ACT drain its exp backlog)
                            if tail_prev is not None:
                                tail_prev[0]()
                            if po_queue:
                                po_queue.pop(0)()
                            if tail_prev is not None:
                                tail_prev[1]()
                            if po_queue:
                                po_queue.pop(0)()
                            if qsb == 0:
                                for _ in range(4):
                                    if transL:
                                        transL.pop(0)()
                            tail_prev = make_tail(h, av, rrh, att, qsb)
                    tail_prev[0]()
                    tail_prev[1]()
                    tail_prev = None
                    if qsb == QSB - 1:
                        # final out-projection: split the last block so its
                        # copy+DMA tail overlaps the second half's matmuls
                        po_queue = [make_po(att, qsb, m) for m in range(KT - 1)]
                        po_queue.append(make_po(att, qsb, KT - 1, 0, 256))
                        po_queue.append(make_po(att, qsb, KT - 1, 256, 256))
                        while po_queue:
                            po_queue.pop(0)()
                    else:
                        po_queue = [make_po(att, qsb, m) for m in range(KT)]

    nc.compile()
    return nc


def _get_nc(causal: bool):
    if causal not in _compiled:
        _compiled[causal] = _build(causal)
    return _compiled[causal]


def kernel(x, freqs_cis, mask, wq, wk, wv, wo):
    import ml_dtypes
    from concourse.bass_utils import run_bass_kernel_spmd

    bf = ml_dtypes.bfloat16
    x = np.asarray(x, dtype=np.float32)
    freqs_cis = np.asarray(freqs_cis, dtype=np.float32)
    mask = np.asarray(mask, dtype=np.float32)
    wq = np.asarray(wq, dtype=np.float32)
    wk = np.asarray(wk, dtype=np.float32)
    wv = np.asarray(wv, dtype=np.float32)
    wo = np.asarray(wo, dtype=np.float32)

    tri = np.tril(np.ones((S, S), dtype=bool))
    causal = bool((mask[tri] == 0.0).all() and (mask[~tri] < -1e30).all())
    if not causal and not (mask == 0.0).all():
        return _numpy_ref(x, freqs_cis, mask, wq, wk, wv, wo)

    nc = _get_nc(causal)

    cos = freqs_cis[:, :, 0]
    sin = freqs_cis[:, :, 1]
    cosS = np.ascontiguousarray(cos.reshape(SB, 128, 64).transpose(1, 0, 2))
    sinS = np.ascontiguousarray(sin.reshape(SB, 128, 64).transpose(1, 0, 2))
    mtile = (np.ascontiguousarray(mask[0:128, 0:128].T) if causal
             else np.zeros((128, 128), dtype=np.float32))
    onest = np.ones((128, 128), dtype=bf)
    basist = np.ascontiguousarray(
        np.broadcast_to(np.eye(4, dtype=bf)[:, :, None], (4, 4, 128)))

    in_maps = []
    for c in range(8):
        b, i = c // 2, c % 2
        # x[b]: [S, D] -> [128 p, SB, KT, 128 j]
        xd = np.ascontiguousarray(
            x[b].reshape(SB, 128, KT, 128).transpose(3, 0, 2, 1).astype(bf))
        wcat = np.concatenate(
            [wq[1024 * i:1024 * (i + 1), :],
             wk[256 * i:256 * (i + 1), :],
             wv[256 * i:256 * (i + 1), :]], axis=0)  # [1536, D]
        wd = np.ascontiguousarray(
            wcat.T.reshape(KT, 128, NE * 128).transpose(1, 0, 2).astype(bf))
        wod = np.ascontiguousarray(
            wo[:, 1024 * i:1024 * (i + 1)].T.reshape(HPC, 128, D)
            .transpose(1, 0, 2).astype(bf))
        in_maps.append({
            "xd": xd, "wd": wd, "wod": wod,
            "cosS": cosS, "sinS": sinS, "mtile": mtile, "onest": onest,
            "basist": basist,
        })

    res = run_bass_kernel_spmd(nc, in_maps, core_ids=list(range(8)))
    out = np.empty((B, S, D), dtype=np.float32)
    for b in range(B):
        out[b] = res.results[2 * b]["outT"].T + res.results[2 * b + 1]["outT"].T
    return out


def _numpy_ref(x, freqs_cis, mask, wq, wk, wv, wo):
    xq = (x @ wq.T).reshape(B, S, H, HD)
    xk = (x @ wk.T).reshape(B, S, KV, HD)
    xv = (x @ wv.T).reshape(B, S, KV, HD)

    def rope(xh):
        x2 = xh.reshape(*xh.shape[:-1], HD // 2, 2)
        fc = freqs_cis[None, :, None, :, :]
        real = x2[..., 0] * fc[..., 0] - x2[..., 1] * fc[..., 1]
        imag = x2[..., 0] * fc[..., 1] + x2[..., 1] * fc[..., 0]
        return np.concatenate([real, imag], axis=-1)

    xq, xk = rope(xq), rope(xk)
    q = xq.reshape(B, S, KV, NREP, HD)
    sc = np.einsum('bqgrd,bkgd->bgrqk', q, xk) * SCALE + mask[None, None, None]
    sc = sc - sc.max(axis=-1, keepdims=True)
    p = np.exp(sc)
    p /= p.sum(axis=-1, keepdims=True)
    o = np.einsum('bgrqk,bkgd->bqgrd', p, xv).reshape(B, S, H * HD)
    return (o @ wo.T).astype(np.float32)


# revision 49
# speedup vs baseline: 1.2851x; 1.0177x over previous
"""Trainium2 Bass kernel for nn_Attention (B=4, S=2048, D=2048, H=16, KV=4, HD=128).

Sharding (8 cores): data-parallel over batch (4) x tensor-parallel over
KV-head-group halves (2). Core c handles batch b=c//2 and q-heads
[8*(c%2), 8*(c%2)+8) == kv groups {2*(c%2), 2*(c%2)+1}. Each core produces a
partial output (its heads' contribution through wo); the host sums the two
partials per batch.

All matmul operands are bf16 (PSUM accumulation stays f32): full PE speed at
any tile width, half the DMA bytes, and 1.0-rate PE transposes. Stage 1 is a
single fused pass over x: per s-block, one PSUM accumulation produces
q(8)+k(2)+v(2) head slots; RoPE is applied in [s, hd] layout, then PE
transposes write qT/kT ([hd, s]); v is copied raw. The transposes for s-block
i are emitted after the matmuls of s-block i+1 so the in-order PE never waits
on the DVE rope. Stage 2 computes attention transposed (scoresT[k,q]; kT
stationary, qT moving) so the ACT exp pass doubles as the PSUM->SBUF move.
Softmax denominators use tiny-output matmuls (probs block as stationary, ones
column moving -> [128q, 1] accumulated over k-blocks) instead of re-streaming
probs through a ones-row matmul; the per-head [128,4] reciprocal is
PE-transposed to [4,128] and broadcast to a [128,512] tile via basis-matrix
matmuls, and normalization is fused into the AV PSUM->SBUF move (one DVE
multiply). Each head's denominator tail is deferred by one head so the PE
never waits on it. wo stays resident in SBUF, and the out-projection of
q-superblock i is interleaved between the attention heads of q-superblock
i+1 (two 128-row output blocks per head), which keeps ACT busy with exps
during what used to be a PE-only out-projection phase.

Hardware notes learned the hard way: PSUM accumulation groups must be
exclusive per bank (interleaved open groups clobber each other on reset), so
the 4-column denominator tile is zero-initialized once per head and every
tiny matmul accumulates with start=False; DVE instructions may read only one
operand from PSUM (the reciprocal-broadcast tile is staged through SBUF); the
Pool engine cannot access PSUM at all. Startup streams sb0+sb1 fused so the
PE rides the 6MB weight DMA, and the last three s-blocks' transposes are
deferred into stage 2's ACT-bound first q-superblock.
"""
import numpy as np

B, S, D = 4, 2048, 2048
H, KV, HD = 16, 4, 128
NREP = H // KV
SCALE = float(HD) ** -0.5

SB = S // 128          # 16 s-blocks
KT = D // 128          # 16 contraction tiles for projections
QSB = S // 512         # 4 q-superblocks
HPC = 8                # q heads per core
GPC = 2                # kv groups per core
NE = HPC + 2 * GPC     # 12 projection head-slots per core (q0..7, k0, k1, v0, v1)
NR = HPC + GPC         # 10 slots that get RoPE

_compiled = {}


def _build(causal: bool):
    import concourse.bass as bass  # noqa: F401
    import concourse.tile as tile
    from concourse import bacc, mybir
    from concourse.masks import make_identity

    f32 = mybir.dt.float32
    bf16 = mybir.dt.bfloat16
    AF = mybir.ActivationFunctionType
    ALU = mybir.AluOpType

    nc = bacc.Bacc("TRN2")

    # x: [128, SB, KT, 128] with x_dram[p, sb, kt, j] = x[b, sb*128+j, kt*128+p]
    xd = nc.dram_tensor("xd", [128, SB, KT, 128], bf16, kind="ExternalInput")
    # fused qkv weights: wt[p, kt, e] = wcat[e, kt*128+p], e over 12*128
    wd = nc.dram_tensor("wd", [128, KT, NE * 128], bf16, kind="ExternalInput")
    # wo: wod[p, h, d] = wo[d, off + h*128 + p]
    wod = nc.dram_tensor("wod", [128, HPC, D], bf16, kind="ExternalInput")
    cosS = nc.dram_tensor("cosS", [128, SB, 64], f32, kind="ExternalInput")
    sinS = nc.dram_tensor("sinS", [128, SB, 64], f32, kind="ExternalInput")
    mtile = nc.dram_tensor("mtile", [128, 128], f32, kind="ExternalInput")
    onest = nc.dram_tensor("onest", [128, 128], bf16, kind="ExternalInput")
    # basis[k, qb, p] = 1.0 if k == qb else 0 (k, qb in 0..3)
    basist = nc.dram_tensor("basist", [4, 4, 128], bf16, kind="ExternalInput")
    outT = nc.dram_tensor("outT", [D, S], f32, kind="ExternalOutput")

    with tile.TileContext(nc) as tc:
        with tc.tile_pool(name="persist", bufs=1) as persist:
            qT = [persist.tile([128, S], bf16, tag=f"qT{h}", name=f"qT{h}") for h in range(HPC)]
            kT = [persist.tile([128, S], bf16, tag=f"kTg{g}", name=f"kTg{g}") for g in range(GPC)]
            vsb = [persist.tile([128, SB, 128], bf16, tag=f"v{g}", name=f"v{g}") for g in range(GPC)]
            msk = persist.tile([128, 128], f32, tag="msk")
            ones = persist.tile([128, 128], bf16, tag="ones")
            zer4 = persist.tile([128, 4], bf16, tag="zer4")
            basis = persist.tile([4, 4, 128], bf16, tag="basis")
            wo_sb = persist.tile([128, HPC, D], bf16, tag="wo")
            ident = persist.tile([128, 128], bf16, tag="ident")
            cos_t = persist.tile([128, SB, 64], f32, tag="cos")
            sin_t = persist.tile([128, SB, 64], f32, tag="sin")
            psb = persist.tile([128, NR, 128], f32, tag="psb")
            psb0 = persist.tile([128, NR, 128], f32, tag="psb0")
            psb1 = persist.tile([128, NR, 128], f32, tag="psb1")
            # late s-blocks' rope output lives in persistent tiles; their
            # transposes are emitted inside stage 2 (fills ACT-bound gaps)
            NLATE = 3
            rpLate = [persist.tile([128, NR, 128], bf16, tag=f"rpL{i}",
                                   name=f"rpL{i}") for i in range(NLATE)]
            # dedicated rope temporaries for the last s-block so the stage-1
            # SBUF pools release as soon as its matmuls finish (stage 2's
            # probs tiles reuse those addresses)
            t1L = persist.tile([128, NR, 64], f32, tag="t1L")
            t2L = persist.tile([128, NR, 64], f32, tag="t2L")

            # ------------ Stage 1: fused projections + RoPE + transposes ----
            s1ctx = tc.tile_pool(name="s1const", bufs=1)
            s1const = s1ctx.__enter__()
            ident_f = s1const.tile([128, 128], f32, tag="identf")
            make_identity(nc, ident_f)
            nc.vector.tensor_copy(out=ident, in_=ident_f)
            nc.vector.tensor_tensor(
                out=zer4, in0=ident[:, 0:4], in1=ident[:, 0:4],
                op=ALU.subtract)
            # preload the ACT exp table so the first real exp (stage 2)
            # doesn't pay the table-load latency
            warm = s1const.tile([1, 8], f32, tag="warm")
            nc.scalar.activation(
                out=warm, in_=ident_f[0:1, 0:8], func=AF.Exp, scale=1.0)

            EW = NE * 128  # 1536
            with tc.tile_pool(name="w1", bufs=1) as wpool, \
                 tc.tile_pool(name="xs1", bufs=3) as xpool, \
                 tc.tile_pool(name="rs1", bufs=3) as rpool, \
                 tc.tile_pool(name="pq1", bufs=2, space="PSUM") as pqp, \
                 tc.tile_pool(name="pt1", bufs=2, space="PSUM") as ptp:
                wt = wpool.tile([128, KT, EW], bf16, tag="wt")

                def make_trans(rp, sb_):
                    def emit():
                        for h in range(NR):
                            pt = ptp.tile([128, 128], bf16, tag="pt")
                            nc.tensor.transpose(pt, rp[:, h, :], ident)
                            dst = qT[h] if h < HPC else kT[h - HPC]
                            nc.vector.tensor_copy(
                                out=dst[:, sb_ * 128:(sb_ + 1) * 128], in_=pt)
                    return emit

                def emit_rope(ps, sb):
                    for g in range(GPC):
                        nc.scalar.copy(
                            out=vsb[g][:, sb, :], in_=ps[:, HPC + GPC + g, :])
                    if sb <= 1 or sb == SB - 1:
                        # copy q/k slots out of PSUM so the PSUM buffer frees
                        # early (sb0/sb1: the fused prefix holds both ps
                        # buffers; sb15: stage 2 reuses the banks)
                        dst_ps = (psb0, psb1, psb)[min(sb, 2)]
                        nc.scalar.copy(out=dst_ps, in_=ps[:, 0:NR, :])
                        src = dst_ps
                    else:
                        src = ps
                    if sb >= SB - NLATE:
                        rp = rpLate[sb - (SB - NLATE)]
                    else:
                        rp = rpool.tile([128, NR, 128], bf16, tag="rope",
                                        name="rope")
                    ev = src[:, 0:NR, 0:128:2]
                    od = src[:, 0:NR, 1:128:2]
                    cb = cos_t[:, None, sb, :].broadcast_to([128, NR, 64])
                    sn = sin_t[:, None, sb, :].broadcast_to([128, NR, 64])
                    if sb == SB - 1:
                        # last s-block: run rope on the idle Pool engine (all
                        # operands are SBUF here) so the DVE queue is clear
                        # for stage 2's zero-inits and mask-adds
                        t1, t2 = t1L, t2L
                        eng = nc.gpsimd
                    else:
                        t1 = rpool.tile([128, NR, 64], f32, tag="t1", name="t1")
                        t2 = rpool.tile([128, NR, 64], f32, tag="t2", name="t2")
                        eng = nc.vector
                    eng.tensor_tensor(out=t1, in0=ev, in1=cb, op=ALU.mult)
                    eng.tensor_tensor(out=t2, in0=od, in1=sn, op=ALU.mult)
                    eng.tensor_tensor(
                        out=rp[:, :, 0:64], in0=t1, in1=t2, op=ALU.subtract)
                    eng.tensor_tensor(out=t1, in0=ev, in1=sn, op=ALU.mult)
                    eng.tensor_tensor(out=t2, in0=od, in1=cb, op=ALU.mult)
                    eng.tensor_tensor(
                        out=rp[:, :, 64:128], in0=t1, in1=t2, op=ALU.add)
                    return rp

                def emit_mm(ps2, xs, kt):
                    for n0 in range(0, EW, 512):
                        nc.tensor.matmul(
                            ps2[:, n0:n0 + 512], xs[:, kt, :],
                            wt[:, kt, n0:n0 + 512],
                            start=(kt == 0), stop=(kt == KT - 1))

                pending = []
                # fused prefix: sb0+sb1 interleaved per weight chunk so the
                # PE has two s-blocks of work while the wt DMA streams in
                xs0 = xpool.tile([128, KT, 128], bf16, tag="xs", name="xs")
                xs1 = xpool.tile([128, KT, 128], bf16, tag="xs", name="xs")
                nc.sync.dma_start(out=xs0[:, 0:4, :], in_=xd[:, 0, 0:4, :])
                nc.sync.dma_start(out=xs1[:, 0:4, :], in_=xd[:, 1, 0:4, :])
                nc.sync.dma_start(out=wt[:, 0, :], in_=wd[:, 0, :])
                nc.sync.dma_start(out=wt[:, 1, :], in_=wd[:, 1, :])
                nc.sync.dma_start(out=wt[:, 2, :], in_=wd[:, 2, :])
                nc.sync.dma_start(out=xs0[:, 4:KT, :], in_=xd[:, 0, 4:KT, :])
                nc.sync.dma_start(out=xs1[:, 4:KT, :], in_=xd[:, 1, 4:KT, :])
                # cos/sin aren't needed until the first rope (~end of the
                # fused prefix): keep every weight chunk ahead of them
                for kt in range(3, KT):
                    nc.sync.dma_start(out=wt[:, kt, :], in_=wd[:, kt, :])
                nc.sync.dma_start(out=cos_t, in_=cosS[:, :, :])
                nc.sync.dma_start(out=sin_t, in_=sinS[:, :, :])
                nc.sync.dma_start(out=msk, in_=mtile[:, :])
                nc.sync.dma_start(out=ones, in_=onest[:, :])
                nc.sync.dma_start(out=basis, in_=basist[:, :, :])
                ps0 = pqp.tile([128, NE, 128], f32, tag="ps", name="ps")
                ps1 = pqp.tile([128, NE, 128], f32, tag="ps", name="ps")
                ps0f = ps0.rearrange("p h d -> p (h d)")
                ps1f = ps1.rearrange("p h d -> p (h d)")
                for kt in range(KT):
                    emit_mm(ps0f, xs0, kt)
                    emit_mm(ps1f, xs1, kt)
                rp0 = emit_rope(ps0, 0)
                rp1 = emit_rope(ps1, 1)
                pending.append(make_trans(rp0, 0))
                pending.append(make_trans(rp1, 1))

                for sb in range(2, SB):
                    xs = xpool.tile([128, KT, 128], bf16, tag="xs", name="xs")
                    nc.sync.dma_start(out=xs, in_=xd[:, sb, :, :])
                    if sb == SB - 1:
                        # wo arrives during the tail of stage 1
                        nc.sync.dma_start(out=wo_sb, in_=wod[:, :, :])
                    ps = pqp.tile([128, NE, 128], f32, tag="ps", name="ps")
                    ps2 = ps.rearrange("p h d -> p (h d)")
                    for kt in range(KT):
                        emit_mm(ps2, xs, kt)
                    # transposes of an earlier s-block (rope long done)
                    if pending:
                        pending.pop(0)()
                    rp = emit_rope(ps, sb)
                    if sb < SB - NLATE:
                        pending.append(make_trans(rp, sb))
                while pending:
                    pending.pop(0)()
            s1ctx.__exit__(None, None, None)

            # ------------ Stage 2: attention (scoresT) + out-projection -----
            with tc.tile_pool(name="pr2", bufs=3) as prpool, \
                 tc.tile_pool(name="att2", bufs=3) as attpool, \
                 tc.tile_pool(name="dn2", bufs=3) as dnpool, \
                 tc.tile_pool(name="o2", bufs=3) as opool, \
                 tc.tile_pool(name="psc", bufs=3, space="PSUM") as pscp, \
                 tc.tile_pool(name="pav", bufs=2, space="PSUM") as pavp, \
                 tc.tile_pool(name="pds", bufs=1, space="PSUM") as pdsp, \
                 tc.tile_pool(name="scr", bufs=2, space="PSUM") as scrp:

                def make_transL(i_, j_, on_act_):
                    def emit():
                        pt = scrp.tile([128, 128], bf16, tag="scr", name="ptL")
                        nc.tensor.transpose(pt, rpLate[i_][:, j_, :], ident)
                        dst = qT[j_] if j_ < HPC else kT[j_ - HPC]
                        sbL = SB - NLATE + i_
                        # alternate the PSUM->SBUF copies between ACT and DVE
                        # to balance the two engines in the ACT-bound qsb0
                        if on_act_:
                            nc.scalar.copy(
                                out=dst[:, sbL * 128:(sbL + 1) * 128], in_=pt)
                        else:
                            nc.vector.tensor_copy(
                                out=dst[:, sbL * 128:(sbL + 1) * 128], in_=pt)
                    return emit

                transL = [make_transL(i, j, (i * NR + j) % 2 == 0)
                          for i in range(NLATE) for j in range(NR)]

                def make_tail(h_, av_, rrh_, att_, qsb_):
                    # part a: transpose the reciprocal row; copy it to SBUF on
                    # DVE (its backlog is far shorter than ACT's exp queue)
                    rrow = [None]

                    def emit_a():
                        trp = scrp.tile([4, 128], bf16, tag="scr")
                        nc.tensor.transpose(trp, rrh_, ident)
                        rrow[0] = dnpool.tile(
                            [4, 128], bf16, tag="rrow", name="rrow")
                        nc.vector.tensor_copy(out=rrow[0], in_=trp)

                    def emit_b():
                        rps = scrp.tile([128, 512], f32, tag="scr")
                        for qb in range(4):
                            nc.tensor.matmul(
                                rps[:, qb * 128:(qb + 1) * 128],
                                basis[:, qb, :], rrow[0],
                                start=True, stop=True)
                        # DVE may read only ONE operand from PSUM: stage the
                        # broadcast tile to SBUF (bf16), then multiply it into
                        # the AV PSUM on the way to the att tile. In qsb0 the
                        # DVE is the regional bottleneck, so stage on ACT there
                        rsb = dnpool.tile([128, 512], bf16, tag="rsb")
                        if qsb_ == 0:
                            nc.scalar.copy(out=rsb, in_=rps)
                        else:
                            nc.vector.tensor_copy(out=rsb, in_=rps)
                        nc.vector.tensor_tensor(
                            out=att_[:, h_, :], in0=av_, in1=rsb,
                            op=ALU.mult)
                    return emit_a, emit_b

                def make_po(att_, qsb_, m_, q0_=0, qw_=512):
                    def emit():
                        po = scrp.tile([128, 512], f32, tag="scr")
                        for e in range(HPC):
                            nc.tensor.matmul(
                                po[:, 0:qw_],
                                wo_sb[:, e, m_ * 128:(m_ + 1) * 128],
                                att_[:, e, q0_:q0_ + qw_],
                                start=(e == 0), stop=(e == HPC - 1))
                        ot = opool.tile([128, 512], f32, tag="ot")
                        nc.vector.tensor_copy(out=ot[:, 0:qw_], in_=po[:, 0:qw_])
                        nc.sync.dma_start(
                            out=outT[m_ * 128:(m_ + 1) * 128,
                                     qsb_ * 512 + q0_:qsb_ * 512 + q0_ + qw_],
                            in_=ot[:, 0:qw_])
                    return emit

                po_queue = []
                tail_prev = None
                for qsb in range(QSB):
                    att = attpool.tile([128, HPC, 512], bf16, tag="att")
                    maxkt = (qsb + 1) * 4 if causal else SB
                    q0g = qsb * 512
                    for g in range(GPC):
                        for r in range(NREP):
                            h = g * NREP + r
                            probs = prpool.tile([128, SB, 512], bf16, tag="probs")
                            dsT = pdsp.tile([128, 4], f32, tag="dsT")
                            av = pavp.tile([128, 512], f32, tag="av")
                            # PSUM accumulation groups must be exclusive per
                            # bank on HW: zero-init the 4-column denominator
                            # tile and accumulate with start=False throughout
                            nc.vector.tensor_copy(out=dsT, in_=zer4)
                            for t in range(maxkt):
                                ql = max(0, t * 128 - q0g) if causal else 0
                                sc = pscp.tile([128, 512], f32, tag="sc")
                                nc.tensor.matmul(
                                    sc[:, ql:512],
                                    kT[g][:, t * 128:(t + 1) * 128],
                                    qT[h][:, q0g + ql:q0g + 512],
                                    start=True, stop=True)
                                is_diag = causal and t * 128 >= q0g
                                if is_diag:
                                    # add mask pre-scale: exp(SCALE*(sc+msk))
                                    # == exp(SCALE*sc + mask) for the 0/-inf
                                    # mask (underflows to 0 identically)
                                    nc.vector.tensor_tensor(
                                        out=sc[:, ql:ql + 128],
                                        in0=sc[:, ql:ql + 128],
                                        in1=msk, op=ALU.add)
                                nc.scalar.activation(
                                    out=probs[:, t, ql:512],
                                    in_=sc[:, ql:512], func=AF.Exp,
                                    scale=SCALE)
                                nc.tensor.matmul(
                                    av[:, ql:512], vsb[g][:, t, :],
                                    probs[:, t, ql:512],
                                    start=(t == 0), stop=(t == maxkt - 1),
                                    skip_group_check=True)
                                # per-q-block denominator partials: tiny-output
                                # matmuls (probs block stationary, ones moving)
                                qb0 = max(0, t - qsb * 4) if causal else 0
                                for qb in range(qb0, 4):
                                    tlast = qsb * 4 + qb if causal else maxkt - 1
                                    nc.tensor.matmul(
                                        dsT[:, qb:qb + 1],
                                        probs[:, t, qb * 128:(qb + 1) * 128],
                                        ones[:, 0:1],
                                        start=False, stop=(t == tlast),
                                        skip_group_check=True)
                            rrh = dnpool.tile([128, 4], bf16, tag="rrh")
                            with nc.allow_low_precision(reason="softmax recip"):
                                nc.vector.reciprocal(out=rrh, in_=dsT)
                            # deferred work: previous head's denominator tail
                            # interleaved with two out-proj blocks of the
                            # previous qsb (the po blocks cover the Pool-copy
                            # latency and let ACT drain its exp backlog)
                            if tail_prev is not None:
                                tail_prev[0]()
                            if po_queue:
                                po_queue.pop(0)()
                            if tail_prev is not None:
                                tail_prev[1]()
                            if po_queue:
                                po_queue.pop(0)()
                            if qsb == 0:
                                for _ in range(4):
                                    if transL:
                                        transL.pop(0)()
                            tail_prev = make_tail(h, av, rrh, att, qsb)
                    tail_prev[0]()
                    tail_prev[1]()
                    tail_prev = None
                    if qsb == QSB - 1:
                        # final out-projection: split the last block so its
                        # copy+DMA tail overlaps the second half's matmuls
                        po_queue = [make_po(att, qsb, m) for m in range(KT - 1)]
                        po_queue.append(make_po(att, qsb, KT - 1, 0, 256))
                        po_queue.append(make_po(att, qsb, KT - 1, 256, 256))
                        while po_queue:
                            po_queue.pop(0)()
                    else:
                        po_queue = [make_po(att, qsb, m) for m in range(KT)]

    nc.compile()
    return nc


def _get_nc(causal: bool):
    if causal not in _compiled:
        _compiled[causal] = _build(causal)
    return _compiled[causal]


def kernel(x, freqs_cis, mask, wq, wk, wv, wo):
    import ml_dtypes
    from concourse.bass_utils import run_bass_kernel_spmd

    bf = ml_dtypes.bfloat16
    x = np.asarray(x, dtype=np.float32)
    freqs_cis = np.asarray(freqs_cis, dtype=np.float32)
    mask = np.asarray(mask, dtype=np.float32)
    wq = np.asarray(wq, dtype=np.float32)
    wk = np.asarray(wk, dtype=np.float32)
    wv = np.asarray(wv, dtype=np.float32)
    wo = np.asarray(wo, dtype=np.float32)

    tri = np.tril(np.ones((S, S), dtype=bool))
    causal = bool((mask[tri] == 0.0).all() and (mask[~tri] < -1e30).all())
    if not causal and not (mask == 0.0).all():
        return _numpy_ref(x, freqs_cis, mask, wq, wk, wv, wo)

    nc = _get_nc(causal)

    cos = freqs_cis[:, :, 0]
    sin = freqs_cis[:, :, 1]
    cosS = np.ascontiguousarray(cos.reshape(SB, 128, 64).transpose(1, 0, 2))
    sinS = np.ascontiguousarray(sin.reshape(SB, 128, 64).transpose(1, 0, 2))
    mtile = (np.ascontiguousarray(mask[0:128, 0:128].T) if causal
             else np.zeros((128, 128), dtype=np.float32))
    onest = np.ones((128, 128), dtype=bf)
    basist = np.ascontiguousarray(
        np.broadcast_to(np.eye(4, dtype=bf)[:, :, None], (4, 4, 128)))

    in_maps = []
    for c in range(8):
        b, i = c // 2, c % 2
        # x[b]: [S, D] -> [128 p, SB, KT, 128 j]
        xd = np.ascontiguousarray(
            x[b].reshape(SB, 128, KT, 128).transpose(3, 0, 2, 1).astype(bf))
        wcat = np.concatenate(
            [wq[1024 * i:1024 * (i + 1), :],
             wk[256 * i:256 * (i + 1), :],
             wv[256 * i:256 * (i + 1), :]], axis=0)  # [1536, D]
        wd = np.ascontiguousarray(
            wcat.T.reshape(KT, 128, NE * 128).transpose(1, 0, 2).astype(bf))
        wod = np.ascontiguousarray(
            wo[:, 1024 * i:1024 * (i + 1)].T.reshape(HPC, 128, D)
            .transpose(1, 0, 2).astype(bf))
        in_maps.append({
            "xd": xd, "wd": wd, "wod": wod,
            "cosS": cosS, "sinS": sinS, "mtile": mtile, "onest": onest,
            "basist": basist,
        })

    res = run_bass_kernel_spmd(nc, in_maps, core_ids=list(range(8)))
    out = np.empty((B, S, D), dtype=np.float32)
    for b in range(B):
        out[b] = res.results[2 * b]["outT"].T + res.results[2 * b + 1]["outT"].T
    return out


def _numpy_ref(x, freqs_cis, mask, wq, wk, wv, wo):
    xq = (x @ wq.T).reshape(B, S, H, HD)
    xk = (x @ wk.T).reshape(B, S, KV, HD)
    xv = (x @ wv.T).reshape(B, S, KV, HD)

    def rope(xh):
        x2 = xh.reshape(*xh.shape[:-1], HD // 2, 2)
        fc = freqs_cis[None, :, None, :, :]
        real = x2[..., 0] * fc[..., 0] - x2[..., 1] * fc[..., 1]
        imag = x2[..., 0] * fc[..., 1] + x2[..., 1] * fc[..., 0]
        return np.concatenate([real, imag], axis=-1)

    xq, xk = rope(xq), rope(xk)
    q = xq.reshape(B, S, KV, NREP, HD)
    sc = np.einsum('bqgrd,bkgd->bgrqk', q, xk) * SCALE + mask[None, None, None]
    sc = sc - sc.max(axis=-1, keepdims=True)
    p = np.exp(sc)
    p /= p.sum(axis=-1, keepdims=True)
    o = np.einsum('bgrqk,bkgd->bqgrd', p, xv).reshape(B, S, H * HD)
    return (o @ wo.T).astype(np.float32)
